# revision 31
# baseline (speedup 1.0000x reference)
"""Trainium2 Bass kernel for batched multi-head attention.

Problem: N=8, S=1024, E=1024, H=16, DK=64 MultiHeadAttention with a boolean
attention mask, fp32 reference.

Strategy: pure batch data-parallelism -- one batch element per NeuronCore
(8 cores), weights replicated, no collectives.  Per core everything is
computed in a transposed layout so no on-chip transposes are needed:

  xT [E, S] (host-transposed)  --Wq/Wk-->  QT, KT [E, S]  (no bias: the
      q-side bias and const cancel in softmax; the k-side bias folds into
      the exp bias B[s,h] = x[s]@(Wk_h bq_h)/8, precomputed on host)
  xT                           --Wv---->   V [S, E] head-major w/ ones col
  logitsT[k, q] = KT_h^T-slices @ QT_h    (K=64 row-paired matmuls on PE
      subarray tiles (0,0)/(64,0) -- two heads' matmuls run concurrently)
  Em = exp(logitsT/8 + B) * notm          (ACT exp w/ per-partition bias,
                                           DVE mask multiply; Em -> SBUF
                                           fp16 ring, 2 pairs deep)
  O_h[d|sum, q] = V_aug_h^T @ Em_h        (CLOSED 8-matmul accumulation
      bursts, delayed one pair: attnV for pair p runs during pair p+1's
      logits window.  Keeping accumulation groups closed around the 64-row
      logits matmuls avoids a ~10x PE penalty when >=2 PSUM groups are
      open across tile-size switches -- measured on HW.)
  oT[e', q] = O_h[0:64] * (1/sums)        (DVE; 1/sums broadcast across
      partitions via DRAM bounce; pair 7 uses a PE broadcast instead)
  out[q, e] = oT^T-slices @ Wo + bo_eff   (fp16 out, host casts to fp32)
"""

import numpy as np
from contextlib import ExitStack

import concourse.bass as bass
import concourse.mybir as mybir
import concourse.tile as tile
from concourse.tile_rust import add_dep_helper
from concourse.vector_clock import ScopedClock
from concourse.bass_utils import run_bass_kernel_spmd

F32 = mybir.dt.float32
F16 = mybir.dt.float16
Exp = mybir.ActivationFunctionType.Exp
MULT = mybir.AluOpType.mult

N, S, E, H, DK = 8, 1024, 1024, 16, 64
P = 128
NT = E // P
NPAIR = H // 2
DKP = DK + 1  # head slot width in V_aug (64 values + ones column)

MM_DT = F16


# ---------------------------------------------------------------------------
# Workaround: this walrus build supports at most ONE semaphore wait per
# instruction.  Split instructions carrying more waits into NOP(wait) chains
# on the same engine, and do the same for the TileContext tail drain.
# ---------------------------------------------------------------------------
_MAXW = 1
_orig_lower = tile.TileContext._lower_ordered_insts
_tilefix_installed = False


def _split_waits(ordered):
    for _bb, insts in ordered.items():
        out = []
        for inst in insts:
            si = inst.sync_info
            if si is not None and len(si.on_wait) > _MAXW:
                waits = list(si.on_wait)
                keep, extra = waits[:_MAXW], waits[_MAXW:]
                for i in range(0, len(extra), _MAXW):
                    out.append(
                        mybir.InstNoOp(
                            name=f"{inst.name}-ws{i}",
                            engine=inst.engine,
                            bass_nofuse=True,
                            sync_info=mybir.SyncInfo(
                                on_wait=extra[i : i + _MAXW], on_update=[]
                            ),
                        )
                    )
                inst.sync_info = mybir.SyncInfo(
                    on_wait=keep, on_update=list(si.on_update)
                )
            out.append(inst)
        insts[:] = out


def _patched_lower(self, ordered):
    _split_waits(ordered)
    return _orig_lower(self, ordered)


def _patched_drain_and_barrier(self, tick_clock, wait_clock):
    nc = self.nc
    drain_inst = nc.sync.drain()
    wait_clock.add_sem_waits(
        drain_inst.ins, ScopedClock({None: tick_clock.global_clock})
    )
    si = drain_inst.ins.sync_info
    waits = list(si.on_wait) if si is not None else []
    if len(waits) > _MAXW:
        drain_inst.ins.sync_info = mybir.SyncInfo(on_wait=[], on_update=[])
        for i in range(0, len(waits), _MAXW):
            nop = nc.sync.nop(nofuse=True)
            nop.ins.sync_info = mybir.SyncInfo(
                on_wait=waits[i : i + _MAXW], on_update=[]
            )
    nc.all_engine_barrier()
    popped = nc._tile_sem_poison_stack.pop()
    assert popped is self._sem_poison
    nc.clear_and_free_semaphores(list(self.sems.allocated().values()))
    nc.all_engine_barrier()


def _install_tilefix():
    global _tilefix_installed
    if not _tilefix_installed:
        tile.TileContext._lower_ordered_insts = _patched_lower
        tile.TileContext._drain_and_barrier = _patched_drain_and_barrier
        _tilefix_installed = True


# ---------------------------------------------------------------------------
# Kernel build
# ---------------------------------------------------------------------------
_cached_nc = {}

# v-proj chunk placement: window -> list of (t, c) emitted as absorbers
_VMAP = {
    0: [(t, 0) for t in range(NT)],
    1: [(0, 1), (1, 1)],
    2: [(2, 1), (3, 1)],
    3: [(4, 1), (5, 1)],
    4: [(6, 1), (7, 1)],
    5: [],
    6: [],
    7: [],
}


def _build(repeat=1):
    if repeat in _cached_nc:
        return _cached_nc[repeat]
    _install_tilefix()

    nc = bass.Bass("TRN2", num_devices=N)

    x_t = nc.declare_dram_parameter("x_t", [E, S], MM_DT, isOutput=False)
    nm_t = nc.declare_dram_parameter("nm_t", [S, S], F16, isOutput=False)
    wq = nc.declare_dram_parameter("wq", [E, E], MM_DT, isOutput=False)
    wk = nc.declare_dram_parameter("wk", [E, E], MM_DT, isOutput=False)
    wv = nc.declare_dram_parameter("wv", [E, E], MM_DT, isOutput=False)
    wo = nc.declare_dram_parameter("wo", [E, E], MM_DT, isOutput=False)
    bt = nc.declare_dram_parameter("bt", [P, NT * H], F32, isOutput=False)
    bo_eff = nc.declare_dram_parameter("bo_eff", [E], F32, isOutput=False)
    out = nc.declare_dram_parameter("out", [S, E], F16, isOutput=True)
    import os as _os
    _dbg = _os.environ.get("KDEBUG") == "1"
    _peonly = _os.environ.get("KPEONLY") == "1"
    _exponly = False
    _expstage = _os.environ.get("KSTAGE") == "exp"
    _maskstage = _os.environ.get("KSTAGE") == "mask"
    if _dbg:
        qt0_d = nc.declare_dram_parameter("qt0_d", [P, S], MM_DT, isOutput=True)
        kt0_d = nc.declare_dram_parameter("kt0_d", [P, S], MM_DT, isOutput=True)
        em_d = nc.declare_dram_parameter("em_d", [P, S], F16, isOutput=True)
        o00_d = nc.declare_dram_parameter("o00_d", [DKP, S], F32, isOutput=True)
        rb00_d = nc.declare_dram_parameter("rb00_d", [DK, S], F32, isOutput=True)
        ot0_d = nc.declare_dram_parameter("ot0_d", [P, S], MM_DT, isOutput=True)
        vg0_d = nc.declare_dram_parameter("vg0_d", [P, H * DKP], F16, isOutput=True)

    def tiled(ap):
        return ap.rearrange("(t p) f -> p t f", p=P)

    x_tt = tiled(x_t.ap())
    nm_tt = tiled(nm_t.ap())
    w_t = {
        "q": tiled(wq.ap()),
        "k": tiled(wk.ap()),
        "v": tiled(wv.ap()),
        "o": tiled(wo.ap()),
    }
    bt_t = bt.ap().rearrange("p (t h) -> p t h", h=H)
    out_t = tiled(out.ap())

    with tile.TileContext(nc) as tc, ExitStack() as ctx:
        p_pers = ctx.enter_context(tc.tile_pool(name="pers", bufs=1))
        p_qk = ctx.enter_context(tc.tile_pool(name="qk", bufs=3))
        QT_p = {}
        KT_p = {}
        oT_t = [p_pers.tile([P, S], MM_DT, name=f"oT{j}") for j in range(NT)]
        nm_all = p_pers.tile([P, NT, S], F16, name="nm_all")
        nm_j = [nm_all[:, j, :] for j in range(NT)]
        Vg = p_pers.tile([P, NT, H * DKP], F16)
        BT_sb = p_pers.tile([P, NT, H], F32)
        ones1 = p_pers.tile([1, DK], F32)
        bo_sb = p_pers.tile([P, S], F32)

        for rep in range(repeat):
            with tc.tile_pool(name="em", bufs=(24 if _dbg else 28)) as p_em, \
                 tc.tile_pool(name="rr", bufs=2) as p_r, \
                 tc.tile_pool(name="rb", bufs=2) as p_rb, \
                 tc.tile_pool(name="po", bufs=2) as p_out, \
                 tc.tile_pool(name="dram", bufs=8, space="DRAM") as p_dram:

                QT_p.clear()
                KT_p.clear()
                nc.gpsimd.memset(Vg[:, :, DK::DKP], 1.0)
                nc.gpsimd.memset(ones1[:], 1.0)

                with tc.tile_pool(name="w", bufs=1) as p_w:
                    xT = p_w.tile([P, NT, S], MM_DT, name=f"xT_{rep}")
                    Wf = {
                        pr: p_w.tile([P, NT, S], MM_DT, name=f"Wf_{rep}_{pr}")
                        for pr in ("q", "k", "v")
                    }
                    # DMA priority order for cold start (batched: HWDGE
                    # costs ~630ns of serialized overhead PER instruction):
                    # SP queue: x + W t0-slices unblock pair-0 projections,
                    # then the Wq/Wk remainders.  ACT queue: mask, V, consts.
                    nc.sync.dma_start(xT[:], x_tt[:])
                    nc.sync.dma_start(Wf["q"][:, :, 0:P], w_t["q"][:, :, 0:P])
                    nc.sync.dma_start(Wf["k"][:, :, 0:P], w_t["k"][:, :, 0:P])
                    nc.scalar.dma_start(BT_sb[:], bt_t[:])
                    nc.scalar.dma_start(nm_all[:], nm_tt[:])
                    nc.sync.dma_start(Wf["q"][:, :, P:S], w_t["q"][:, :, P:S])
                    nc.sync.dma_start(Wf["k"][:, :, P:S], w_t["k"][:, :, P:S])
                    nc.scalar.dma_start(
                        Wf["v"][:, :, 0:512], w_t["v"][:, :, 0:512]
                    )
                    nc.scalar.dma_start(
                        Wf["v"][:, :, 512:1024], w_t["v"][:, :, 512:1024]
                    )
                    nc.scalar.dma_start(
                        bo_sb[:],
                        bo_eff.ap().rearrange("(o e) -> o e", o=1).broadcast_to((P, S)),
                    )

                    with tc.tile_pool(name="psO", bufs=2, space="PSUM") as psO:
                        psL = tc.alloc_tile_pool(name="psL", bufs=4, space="PSUM")

                        _pe_prev = [None]
                        _vg_copy = {}

                        def pe_mm(*args, chain=True, **kw):
                            # pin PE program order to emission order: the
                            # list scheduler otherwise hoists matmuls by
                            # readiness, splitting accumulation bursts
                            # around 64-row tiled matmuls (a ~10x HW
                            # penalty when >=2 PSUM groups are open across
                            # a tile-size switch) and racing ahead of Vg
                            # writers whose strided copies it won't track.
                            h = nc.tensor.matmul(*args, **kw)
                            if chain:
                                if _pe_prev[0] is not None:
                                    add_dep_helper(
                                        h.ins, _pe_prev[0], sync=False,
                                        reason="pe absorber-unit order",
                                    )
                                _pe_prev[0] = h.ins
                            return h

                        def emit_proj_chunk(proj, t, c):
                            acc = psL.tile(
                                [P, 512], F32, tag="L",
                                name=f"a2_{rep}_{proj}_{t}_{c}",
                            )
                            for j in range(NT):
                                wt = Wf[proj][:, j, :]
                                if proj == "v":
                                    lhsT = xT[:, j, t * P : (t + 1) * P]
                                    rhs = wt[:, c * 512 : (c + 1) * 512]
                                else:
                                    lhsT = wt[:, t * P : (t + 1) * P]
                                    rhs = xT[:, j, c * 512 : (c + 1) * 512]
                                pe_mm(
                                    acc[:], lhsT, rhs,
                                    start=(j == 0), stop=(j == NT - 1),
                                )
                            if proj == "v":
                                dst = Vg[
                                    :, t, c * 8 * DKP : (c + 1) * 8 * DKP
                                ].rearrange("p (h d) -> p h d", d=DKP)[:, :, 0:DK]
                                src = acc[:].rearrange("p (h d) -> p h d", d=DK)
                                cp = nc.vector.tensor_copy(dst, src)
                                _vg_copy[(t, c)] = cp.ins
                            else:
                                dst, tg = (
                                    (QT_p, "QT") if proj == "q" else (KT_p, "KT")
                                )
                                if t not in dst:
                                    dst[t] = p_qk.tile(
                                        [P, S], MM_DT, tag=tg,
                                        name=f"{tg}_{rep}_{t}",
                                    )
                                nc.vector.tensor_copy(
                                    dst[t][:, c * 512 : (c + 1) * 512], acc[:]
                                )

                        em_tiles = {}

                        def emit_logits_exp(p, j):
                            # four 1-bank L units per quartet; exps split per
                            # (h, c-half) so psL units release every ~570ns
                            # instead of per 2us exp pair (shrinks the
                            # PE<->ACT round-trip lockstep)
                            L = {}
                            for c in range(2):
                                for h in range(2):
                                    r0, r1 = h * DK, (h + 1) * DK
                                    L[(h, c)] = psL.tile(
                                        [P, 512], F32, tag="L",
                                        name=f"L_{rep}_{p}_{j}_{h}_{c}",
                                    )
                                    pe_mm(
                                        L[(h, c)][:],
                                        KT_p[p][r0:r1, j * P : (j + 1) * P],
                                        QT_p[p][r0:r1, c * 512 : (c + 1) * 512],
                                        start=True, stop=True,
                                        tile_position=(r0, 0),
                                        chain=False,
                                    )
                            if _peonly:
                                for h in range(2):
                                    nc.vector.tensor_copy(
                                        oT_t[0][0:1, j * 16 : j * 16 + 16],
                                        L[(h, 0)][0:1, 0:16],
                                    )
                                return
                            ems = {}
                            for h in range(2):
                                ems[h] = p_em.tile(
                                    [P, S], F16,
                                    tag=("Eme" if _expstage else "Em"),
                                    name=f"Em_{rep}_{p}_{j}_{h}",
                                    bufs=(3 if _expstage else None),
                                )
                                em_tiles[(p, j, h)] = ems[h]
                            import os as _os
                            for c in range(2):
                                for h in range(2):
                                    _bias = (
                                        0.0 if _os.environ.get("KBIAS") == "0"
                                        else BT_sb[:, j, 2 * p + h : 2 * p + h + 1]
                                    )
                                    nc.scalar.activation(
                                        ems[h][:, c * 512 : (c + 1) * 512],
                                        L[(h, c)][:], Exp, scale=0.125,
                                        bias=_bias,
                                    )
                            if _expstage:
                                nc.vector.tensor_copy(
                                    oT_t[1][0:1, 0:16], ems[0][0:1, 0:16],
                                )

                        def emit_mask(p, j):
                            if _expstage:
                                return
                            for h in range(2):
                                em = em_tiles[(p, j, h)]
                                eng = (
                                    nc.gpsimd
                                    if h == 0 and j in (1, 3, 5, 6)
                                    else nc.vector
                                )
                                eng.tensor_tensor(
                                    em[:], em[:], nm_j[j][:], MULT,
                                )
                                if _dbg and rep == 0 and (p, j, h) == (0, 0, 0):
                                    nc.sync.dma_start(em_d.ap(), em[:])

                        O_cur = {}

                        def emit_attn_burst(p, h, c):
                            # closed 8-matmul accumulation burst for pair p
                            if (p, h) not in O_cur:
                                O_cur[(p, h)] = psO.tile(
                                    [DKP, S], F32, tag="O",
                                    name=f"O_{rep}_{p}_{h}",
                                )
                            O = O_cur[(p, h)]
                            head = 2 * p + h
                            chalf = head // 8
                            jseq = [NT - 1] + list(range(NT - 1))
                            for i, j in enumerate(jseq):
                                rhs = (
                                    nm_j[j][:, c * 512 : (c + 1) * 512]
                                    if (_peonly or _expstage) else
                                    em_tiles[(p, j, h)][:, c * 512 : (c + 1) * 512]
                                )
                                mm = pe_mm(
                                    O[:, c * 512 : (c + 1) * 512],
                                    Vg[:, j, head * DKP : (head + 1) * DKP],
                                    rhs,
                                    start=(i == 0), stop=(i == NT - 1),
                                )
                                if i == 0:
                                    # the tile framework does not track the
                                    # strided Vg writes; sync the burst on
                                    # the last v-chunk copy of its c-half
                                    dep = _vg_copy.get((NT - 1, chalf))
                                    if dep is not None:
                                        add_dep_helper(
                                            mm.ins, dep, sync=True,
                                            reason="Vg strided-write race",
                                        )

                        def emit_norm_a(p, h):
                            if _peonly or _expstage or _maskstage:
                                nc.vector.tensor_copy(
                                    oT_t[p][0:DKP, 0:16],
                                    O_cur.pop((p, h))[:, 0:16],
                                )
                                return None
                            # reciprocal of both heads' softmax sums into one
                            # [2, S] tile; one DRAM-bounce broadcast to
                            # [2*DK, S] per pair (HWDGE instruction count)
                            O = O_cur[(p, h)]
                            if p == NPAIR - 1:
                                R7 = p_r.tile(
                                    [1, S], F32, tag="R", name=f"R7_{rep}_{h}"
                                )
                                nc.vector.reciprocal(R7[:], O[DK : DK + 1, :])
                                return R7
                            if h == 0:
                                R2 = p_r.tile(
                                    [2 * 32, S], F16, tag="R",
                                    name=f"R_{rep}_{p}",
                                )
                                emit_norm_a.r2 = R2
                            else:
                                R2 = emit_norm_a.r2
                            with nc.allow_low_precision(
                                reason="1/softmax-sum bounce in fp16; 5e-4 "
                                "rel err on normalized weights is fine"
                            ):
                                nc.vector.reciprocal(
                                    R2[32 * h : 32 * h + 1, :],
                                    O[DK : DK + 1, :],
                                )
                            if h == 0:
                                return None
                            Rd = p_dram.tile(
                                [2, S], F16, tag="Rd", name=f"Rd_{rep}_{p}"
                            )
                            nc.scalar.dma_start(
                                Rd[:],
                                R2[:].rearrange(
                                    "(a b) f -> a b f", b=32
                                )[:, 0:1, :],
                            )
                            Rb = p_rb.tile(
                                [2 * DK, S], F16, tag="Rb", name=f"Rb_{rep}_{p}"
                            )
                            nc.scalar.dma_start(
                                Rb[0:DK, :], Rd[0:1, :].broadcast_to((DK, S))
                            )
                            nc.scalar.dma_start(
                                Rb[DK : 2 * DK, :],
                                Rd[1:2, :].broadcast_to((DK, S)),
                            )
                            return Rb

                        def emit_norm_b(p, h, Rb):
                            if _peonly or _expstage or _maskstage:
                                return
                            O = O_cur.pop((p, h))
                            if _dbg and rep == 0 and (p, h) == (0, 0):
                                oc = p_rb.tile([DKP, S], F32, tag="dbg",
                                               name="ocdbg")
                                nc.vector.tensor_copy(oc[:], O[:])
                                nc.sync.dma_start(o00_d.ap(), oc[:])
                                nc.sync.dma_start(rb00_d.ap(), Rb[:])
                            Rbh = (
                                Rb[h * DK : (h + 1) * DK, :]
                                if Rb.partition_size() == 2 * DK else Rb[:]
                            )
                            nc.vector.tensor_tensor(
                                oT_t[p][h * DK : (h + 1) * DK, :],
                                O[0:DK, :], Rbh[:], MULT,
                            )

                        # ---- head: pair-0 q/k projections ----
                        for c in range(2):
                            emit_proj_chunk("q", 0, c)
                        for c in range(2):
                            emit_proj_chunk("k", 0, c)

                        if _dbg:
                            nc.sync.dma_start(qt0_d.ap(), QT_p[0][:])
                            nc.sync.dma_start(kt0_d.ap(), KT_p[0][:])
                        # ---- pair windows ----
                        norm_rb = {}
                        for p in range(NPAIR):
                            if p == NPAIR - 1:
                                # Wq is dead; reuse its SBUF for Wo.
                                nc.scalar.dma_start(
                                    Wf["q"][:], w_t["o"][:]
                                )
                            # absorber queue for this window
                            absq = []
                            if p > 0:
                                for h, c in ((0, 0), (0, 1), (1, 0), (1, 1)):
                                    absq.append(("burst", p - 1, h, c))
                            if p < NPAIR - 1:
                                for pr in ("q", "k"):
                                    for c in range(2):
                                        absq.append(("chunk", pr, p + 1, c))
                            for t, c in _VMAP[p]:
                                absq.append(("chunk", "v", t, c))
                            # interleave: spread absorbers evenly across the
                            # 8 quartet slots (emit after each quartet until
                            # the backlog matches the remaining slots)
                            total = len(absq)

                            def pop_abs():
                                op = absq.pop(0)
                                if op[0] == "burst":
                                    _, bp, bh, bc = op
                                    emit_attn_burst(bp, bh, bc)
                                    if bc == 1:
                                        rbv = emit_norm_a(bp, bh)
                                        if rbv is not None:
                                            for hh in range(2):
                                                emit_norm_b(bp, hh, rbv)
                                else:
                                    _, pr, t, c = op
                                    emit_proj_chunk(pr, t, c)

                            for j in range(NT):
                                emit_logits_exp(p, j)
                                want_done = (total * (j + 1) + NT - 1) // NT
                                while absq and total - len(absq) < want_done:
                                    pop_abs()
                                if j >= 2:
                                    emit_mask(p, j - 2)
                            while absq:
                                pop_abs()
                            emit_mask(p, NT - 2)
                            emit_mask(p, NT - 1)


                        if _dbg:
                            nc.sync.dma_start(ot0_d.ap(), oT_t[0][:])
                            nc.sync.dma_start(vg0_d.ap(), Vg[:, 0, :])
                        # ---- tail: attnV(7), PE-broadcast norm, out-proj ----
                        R_hist = {}
                        for h, c in ((0, 0), (0, 1), (1, 0), (1, 1)):
                            emit_attn_burst(NPAIR - 1, h, c)
                            if c == 1:
                                R_hist[h] = emit_norm_a(NPAIR - 1, h)
                        psL.release()
                        with tc.tile_pool(name="psC", bufs=2, space="PSUM") as psC:
                            p7 = NPAIR - 1
                            if _peonly or _expstage or _maskstage:
                                nc.gpsimd.memset(oT_t[0][:], 0.5)
                                nc.gpsimd.memset(oT_t[1][:, 16:S], 0.5)
                                for jj in range(1, NT):
                                    nc.gpsimd.memset(oT_t[jj][:], 0.5)
                            for h in range(2 * (0 if (_peonly or _expstage or _maskstage) else 1)):
                                Rp = psC.tile(
                                    [DK, S], F32, tag="F", name=f"Rp_{rep}_{h}"
                                )
                                for c in range(2):
                                    pe_mm(
                                        Rp[:, c * 512 : (c + 1) * 512],
                                        ones1[:],
                                        R_hist[h][:, c * 512 : (c + 1) * 512],
                                        start=True, stop=True,
                                    )
                                Rs = p_rb.tile(
                                    [DK, S], F32, tag="Rb", name=f"Rs_{rep}_{h}"
                                )
                                nc.vector.tensor_copy(Rs[:], Rp[:])
                                emit_norm_b(p7, h, Rs)
                            for t in range(NT):
                                F = psC.tile(
                                    [P, S], F32, tag="F", name=f"F_{rep}_{t}"
                                )
                                for j in range(NT):
                                    for c in range(2):
                                        pe_mm(
                                            F[:, c * 512 : (c + 1) * 512],
                                            oT_t[j][:, t * P : (t + 1) * P],
                                            Wf["q"][:, j, c * 512 : (c + 1) * 512],
                                            start=(j == 0), stop=(j == NT - 1),
                                        )
                                ot = p_out.tile(
                                    [P, S], F16, tag="ot", name=f"ot_{rep}_{t}"
                                )
                                nc.vector.tensor_add(ot[:], F[:], bo_sb[:])
                                nc.sync.dma_start(out_t[:, t, :], ot[:])

    _cached_nc[repeat] = nc
    return nc


# ---------------------------------------------------------------------------
# Entry point
# ---------------------------------------------------------------------------
def make_in_maps(x, attn_mask, Wq, bq, Wk, bk, Wv, bv, Wo, bo):
    ndt = np.float16
    Wk64 = np.asarray(Wk, np.float64)
    bq64 = np.asarray(bq, np.float64)
    # per-head exp bias direction: wb[:, h] = (Wk_h @ bq_h) / 8
    WB = np.stack(
        [
            Wk64[:, h * DK : (h + 1) * DK] @ bq64[h * DK : (h + 1) * DK] / 8.0
            for h in range(H)
        ],
        axis=1,
    )  # [E, H]
    bo_eff = (
        np.asarray(bv, np.float64) @ np.asarray(Wo, np.float64)
        + np.asarray(bo, np.float64)
    ).astype(np.float32)
    wqc = np.asarray(Wq, np.float32).astype(ndt)
    wkc = np.asarray(Wk, np.float32).astype(ndt)
    wvc = np.asarray(Wv, np.float32).astype(ndt)
    woc = np.asarray(Wo, np.float32).astype(ndt)
    in_maps = []
    for n in range(N):
        notm_t = np.ascontiguousarray(
            (1.0 - np.asarray(attn_mask[n], np.float32)).T
        ).astype(np.float16)
        B = np.asarray(x[n], np.float64) @ WB  # [S, H]
        btc = np.ascontiguousarray(
            B.reshape(NT, P, H).transpose(1, 0, 2).reshape(P, NT * H)
        ).astype(np.float32)
        in_maps.append(
            {
                "x_t": np.ascontiguousarray(np.asarray(x[n], np.float32).T).astype(ndt),
                "nm_t": notm_t,
                "wq": wqc, "wk": wkc, "wv": wvc, "wo": woc,
                "bt": btc, "bo_eff": bo_eff,
            }
        )
    return in_maps


def kernel(x, attn_mask, Wq, bq, Wk, bk, Wv, bv, Wo, bo, **_):
    nc = _build()
    in_maps = make_in_maps(x, attn_mask, Wq, bq, Wk, bk, Wv, bv, Wo, bo)
    res = run_bass_kernel_spmd(nc, in_maps, list(range(N)))
    outs = np.stack([np.asarray(res.results[n]["out"]) for n in range(N)], axis=0)
    return outs.astype(np.float32)


# revision 36
# speedup vs baseline: 1.1020x; 1.1020x over previous
"""Trainium2 Bass kernel for batched multi-head attention.

Problem: N=8, S=1024, E=1024, H=16, DK=64 MultiHeadAttention with a boolean
attention mask, fp32 reference.

Strategy: pure batch data-parallelism -- one batch element per NeuronCore
(8 cores), weights replicated, no collectives.  Per core everything is
computed in a transposed layout so no on-chip transposes are needed:

  xT [E, S] (host-transposed)  --Wq/Wk-->  QT, KT [E, S]  (no bias: the
      q-side bias and const cancel in softmax; the k-side bias folds into
      the exp bias B[s,h] = x[s]@(Wk_h bq_h)/8, precomputed on host)
  xT                           --Wv---->   V [S, E] head-major w/ ones col
  logitsT[k, q] = KT_h^T-slices @ QT_h    (K=64 row-paired matmuls on PE
      subarray tiles (0,0)/(64,0) -- two heads' matmuls run concurrently)
  Em = exp(logitsT/8 + B) * notm          (ACT exp w/ per-partition bias,
                                           DVE mask multiply; Em -> SBUF
                                           fp16 ring, 2 pairs deep)
  O_h[d|sum, q] = V_aug_h^T @ Em_h        (CLOSED 8-matmul accumulation
      bursts, delayed one pair: attnV for pair p runs during pair p+1's
      logits window.  Keeping accumulation groups closed around the 64-row
      logits matmuls avoids a ~10x PE penalty when >=2 PSUM groups are
      open across tile-size switches -- measured on HW.)
  oT[e', q] = O_h[0:64] * (1/sums)        (DVE; 1/sums broadcast across
      partitions via DRAM bounce; pair 7 uses a PE broadcast instead)
  out[q, e] = oT^T-slices @ Wo + bo_eff   (fp16 out, host casts to fp32)
"""

import numpy as np
from contextlib import ExitStack

import concourse.bass as bass
import concourse.mybir as mybir
import concourse.tile as tile
from concourse.tile_rust import add_dep_helper
from concourse.vector_clock import ScopedClock
from concourse.bass_utils import run_bass_kernel_spmd

F32 = mybir.dt.float32
F16 = mybir.dt.float16
Exp = mybir.ActivationFunctionType.Exp
MULT = mybir.AluOpType.mult

N, S, E, H, DK = 8, 1024, 1024, 16, 64
P = 128
NT = E // P
NPAIR = H // 2
DKP = DK + 1  # head slot width in V_aug (64 values + ones column)

MM_DT = F16


# ---------------------------------------------------------------------------
# Workaround: this walrus build supports at most ONE semaphore wait per
# instruction.  Split instructions carrying more waits into NOP(wait) chains
# on the same engine, and do the same for the TileContext tail drain.
# ---------------------------------------------------------------------------
_MAXW = 1
_orig_lower = tile.TileContext._lower_ordered_insts
_tilefix_installed = False


def _split_waits(ordered):
    for _bb, insts in ordered.items():
        out = []
        for inst in insts:
            si = inst.sync_info
            if si is not None and len(si.on_wait) > _MAXW:
                waits = list(si.on_wait)
                keep, extra = waits[:_MAXW], waits[_MAXW:]
                for i in range(0, len(extra), _MAXW):
                    out.append(
                        mybir.InstNoOp(
                            name=f"{inst.name}-ws{i}",
                            engine=inst.engine,
                            bass_nofuse=True,
                            sync_info=mybir.SyncInfo(
                                on_wait=extra[i : i + _MAXW], on_update=[]
                            ),
                        )
                    )
                inst.sync_info = mybir.SyncInfo(
                    on_wait=keep, on_update=list(si.on_update)
                )
            out.append(inst)
        insts[:] = out


def _patched_lower(self, ordered):
    _split_waits(ordered)
    return _orig_lower(self, ordered)


def _patched_drain_and_barrier(self, tick_clock, wait_clock):
    nc = self.nc
    drain_inst = nc.sync.drain()
    wait_clock.add_sem_waits(
        drain_inst.ins, ScopedClock({None: tick_clock.global_clock})
    )
    si = drain_inst.ins.sync_info
    waits = list(si.on_wait) if si is not None else []
    if len(waits) > _MAXW:
        drain_inst.ins.sync_info = mybir.SyncInfo(on_wait=[], on_update=[])
        for i in range(0, len(waits), _MAXW):
            nop = nc.sync.nop(nofuse=True)
            nop.ins.sync_info = mybir.SyncInfo(
                on_wait=waits[i : i + _MAXW], on_update=[]
            )
    nc.all_engine_barrier()
    popped = nc._tile_sem_poison_stack.pop()
    assert popped is self._sem_poison
    nc.clear_and_free_semaphores(list(self.sems.allocated().values()))
    nc.all_engine_barrier()


def _install_tilefix():
    global _tilefix_installed
    if not _tilefix_installed:
        tile.TileContext._lower_ordered_insts = _patched_lower
        tile.TileContext._drain_and_barrier = _patched_drain_and_barrier
        _tilefix_installed = True


# ---------------------------------------------------------------------------
# Kernel build
# ---------------------------------------------------------------------------
_cached_nc = {}

# v-proj chunk placement: window -> list of (t, c) emitted as absorbers
_VMAP = {
    0: [(t, 0) for t in range(NT)],
    1: [(0, 1), (1, 1)],
    2: [(2, 1), (3, 1)],
    3: [(4, 1), (5, 1)],
    4: [(6, 1), (7, 1)],
    5: [],
    6: [],
    7: [],
}


def _build(repeat=1):
    if repeat in _cached_nc:
        return _cached_nc[repeat]
    _install_tilefix()

    nc = bass.Bass("TRN2", num_devices=N)

    x_t = nc.declare_dram_parameter("x_t", [E, S], MM_DT, isOutput=False)
    nm_t = nc.declare_dram_parameter("nm_t", [S, S], F16, isOutput=False)
    wq = nc.declare_dram_parameter("wq", [E, E], MM_DT, isOutput=False)
    wk = nc.declare_dram_parameter("wk", [E, E], MM_DT, isOutput=False)
    wv = nc.declare_dram_parameter("wv", [E, E], MM_DT, isOutput=False)
    wo = nc.declare_dram_parameter("wo", [E, E], MM_DT, isOutput=False)
    bt = nc.declare_dram_parameter("bt", [P, NT * H], F32, isOutput=False)
    bo_eff = nc.declare_dram_parameter("bo_eff", [E], F32, isOutput=False)
    out = nc.declare_dram_parameter("out", [S, E], F16, isOutput=True)
    import os as _os
    _dbg = _os.environ.get("KDEBUG") == "1"
    _peonly = _os.environ.get("KPEONLY") == "1"
    _exponly = False
    _expstage = _os.environ.get("KSTAGE") == "exp"
    _maskstage = _os.environ.get("KSTAGE") == "mask"
    if _dbg:
        qt0_d = nc.declare_dram_parameter("qt0_d", [P, S], MM_DT, isOutput=True)
        kt0_d = nc.declare_dram_parameter("kt0_d", [P, S], MM_DT, isOutput=True)
        em_d = nc.declare_dram_parameter("em_d", [P, S], F16, isOutput=True)
        o00_d = nc.declare_dram_parameter("o00_d", [DKP, S], F32, isOutput=True)
        rb00_d = nc.declare_dram_parameter("rb00_d", [DK, S], F32, isOutput=True)
        ot0_d = nc.declare_dram_parameter("ot0_d", [P, S], MM_DT, isOutput=True)
        vg0_d = nc.declare_dram_parameter("vg0_d", [P, H * DKP], F16, isOutput=True)

    def tiled(ap):
        return ap.rearrange("(t p) f -> p t f", p=P)

    x_tt = tiled(x_t.ap())
    nm_tt = tiled(nm_t.ap())
    w_t = {
        "q": tiled(wq.ap()),
        "k": tiled(wk.ap()),
        "v": tiled(wv.ap()),
        "o": tiled(wo.ap()),
    }
    bt_t = bt.ap().rearrange("p (t h) -> p t h", h=H)
    out_t = tiled(out.ap())

    with tile.TileContext(nc) as tc, ExitStack() as ctx:
        p_pers = ctx.enter_context(tc.tile_pool(name="pers", bufs=1))
        p_qk = ctx.enter_context(tc.tile_pool(name="qk", bufs=3))
        QT_p = {}
        KT_p = {}
        oT_t = [p_pers.tile([P, S], MM_DT, name=f"oT{j}") for j in range(NT)]
        nm_all = p_pers.tile([P, NT, S], F16, name="nm_all")
        nm_j = [nm_all[:, j, :] for j in range(NT)]
        Vg = p_pers.tile([P, NT, H * DKP], F16)
        BT_sb = p_pers.tile([P, NT, H], F32)
        ones1 = p_pers.tile([1, DK], F32)
        bo_sb = p_pers.tile([P, S], F32)

        for rep in range(repeat):
            with tc.tile_pool(name="em", bufs=(22 if _dbg else 24)) as p_em, \
                 tc.tile_pool(name="rr", bufs=2) as p_r, \
                 tc.tile_pool(name="rb", bufs=3) as p_rb, \
                 tc.tile_pool(name="oc", bufs=3) as p_oc, \
                 tc.tile_pool(name="po", bufs=2) as p_out, \
                 tc.tile_pool(name="dram", bufs=8, space="DRAM") as p_dram:

                QT_p.clear()
                KT_p.clear()
                nc.gpsimd.memset(Vg[:, :, DK::DKP], 1.0)
                nc.gpsimd.memset(ones1[:], 1.0)

                with tc.tile_pool(name="w", bufs=1) as p_w:
                    xT = p_w.tile([P, NT, S], MM_DT, name=f"xT_{rep}")
                    Wf = {
                        pr: p_w.tile([P, NT, S], MM_DT, name=f"Wf_{rep}_{pr}")
                        for pr in ("q", "k", "v")
                    }
                    # DMA priority order for cold start (batched: HWDGE
                    # costs ~630ns of serialized overhead PER instruction):
                    # SP queue: x + W t0-slices unblock pair-0 projections,
                    # then the Wq/Wk remainders.  ACT queue: mask, V, consts.
                    nc.sync.dma_start(xT[:], x_tt[:])
                    nc.sync.dma_start(Wf["q"][:, :, 0:P], w_t["q"][:, :, 0:P])
                    nc.sync.dma_start(Wf["k"][:, :, 0:P], w_t["k"][:, :, 0:P])
                    nc.scalar.dma_start(BT_sb[:], bt_t[:])
                    nc.scalar.dma_start(nm_all[:], nm_tt[:])
                    nc.sync.dma_start(Wf["q"][:, :, P:S], w_t["q"][:, :, P:S])
                    nc.sync.dma_start(Wf["k"][:, :, P:S], w_t["k"][:, :, P:S])
                    nc.scalar.dma_start(
                        Wf["v"][:, :, 0:512], w_t["v"][:, :, 0:512]
                    )
                    nc.scalar.dma_start(
                        Wf["v"][:, :, 512:1024], w_t["v"][:, :, 512:1024]
                    )
                    nc.scalar.dma_start(
                        bo_sb[:],
                        bo_eff.ap().rearrange("(o e) -> o e", o=1).broadcast_to((P, S)),
                    )

                    with tc.tile_pool(name="psO", bufs=2, space="PSUM") as psO:
                        psL = tc.alloc_tile_pool(name="psL", bufs=4, space="PSUM")

                        _pe_prev = [None]
                        _vg_copy = {}

                        def pe_mm(*args, chain=True, **kw):
                            # pin PE program order to emission order: the
                            # list scheduler otherwise hoists matmuls by
                            # readiness, splitting accumulation bursts
                            # around 64-row tiled matmuls (a ~10x HW
                            # penalty when >=2 PSUM groups are open across
                            # a tile-size switch) and racing ahead of Vg
                            # writers whose strided copies it won't track.
                            h = nc.tensor.matmul(*args, **kw)
                            if chain:
                                if _pe_prev[0] is not None:
                                    add_dep_helper(
                                        h.ins, _pe_prev[0], sync=False,
                                        reason="pe absorber-unit order",
                                    )
                                _pe_prev[0] = h.ins
                            return h

                        def emit_proj_chunk(proj, t, c):
                            acc = psL.tile(
                                [P, 512], F32, tag="L",
                                name=f"a2_{rep}_{proj}_{t}_{c}",
                            )
                            for j in range(NT):
                                wt = Wf[proj][:, j, :]
                                if proj == "v":
                                    lhsT = xT[:, j, t * P : (t + 1) * P]
                                    rhs = wt[:, c * 512 : (c + 1) * 512]
                                else:
                                    lhsT = wt[:, t * P : (t + 1) * P]
                                    rhs = xT[:, j, c * 512 : (c + 1) * 512]
                                pe_mm(
                                    acc[:], lhsT, rhs,
                                    start=(j == 0), stop=(j == NT - 1),
                                )
                            if proj == "v":
                                dst = Vg[
                                    :, t, c * 8 * DKP : (c + 1) * 8 * DKP
                                ].rearrange("p (h d) -> p h d", d=DKP)[:, :, 0:DK]
                                src = acc[:].rearrange("p (h d) -> p h d", d=DK)
                                cp = nc.vector.tensor_copy(dst, src)
                                _vg_copy[(t, c)] = cp.ins
                            else:
                                dst, tg = (
                                    (QT_p, "QT") if proj == "q" else (KT_p, "KT")
                                )
                                if t not in dst:
                                    dst[t] = p_qk.tile(
                                        [P, S], MM_DT, tag=tg,
                                        name=f"{tg}_{rep}_{t}",
                                    )
                                nc.vector.tensor_copy(
                                    dst[t][:, c * 512 : (c + 1) * 512], acc[:]
                                )

                        em_tiles = {}

                        def emit_logits_exp(p, j):
                            # four 1-bank L units per quartet; exps split per
                            # (h, c-half) so psL units release every ~570ns
                            # instead of per 2us exp pair (shrinks the
                            # PE<->ACT round-trip lockstep)
                            L = {}
                            for c in range(2):
                                for h in range(2):
                                    r0, r1 = h * DK, (h + 1) * DK
                                    L[(h, c)] = psL.tile(
                                        [P, 512], F32, tag="L",
                                        name=f"L_{rep}_{p}_{j}_{h}_{c}",
                                    )
                                    pe_mm(
                                        L[(h, c)][:],
                                        KT_p[p][r0:r1, j * P : (j + 1) * P],
                                        QT_p[p][r0:r1, c * 512 : (c + 1) * 512],
                                        start=True, stop=True,
                                        tile_position=(r0, 0),
                                        chain=False,
                                    )
                            if _peonly:
                                for h in range(2):
                                    nc.vector.tensor_copy(
                                        oT_t[0][0:1, j * 16 : j * 16 + 16],
                                        L[(h, 0)][0:1, 0:16],
                                    )
                                return
                            ems = {}
                            for h in range(2):
                                ems[h] = p_em.tile(
                                    [P, S], F16,
                                    tag=("Eme" if _expstage else "Em"),
                                    name=f"Em_{rep}_{p}_{j}_{h}",
                                    bufs=(3 if _expstage else None),
                                )
                                em_tiles[(p, j, h)] = ems[h]
                            import os as _os
                            for c in range(2):
                                for h in range(2):
                                    _bias = (
                                        0.0 if _os.environ.get("KBIAS") == "0"
                                        else BT_sb[:, j, 2 * p + h : 2 * p + h + 1]
                                    )
                                    nc.scalar.activation(
                                        ems[h][:, c * 512 : (c + 1) * 512],
                                        L[(h, c)][:], Exp, scale=0.125,
                                        bias=_bias,
                                    )
                            if _expstage:
                                nc.vector.tensor_copy(
                                    oT_t[1][0:1, 0:16], ems[0][0:1, 0:16],
                                )

                        def emit_mask(p, j):
                            if _expstage:
                                return
                            for h in range(2):
                                em = em_tiles[(p, j, h)]
                                eng = (
                                    nc.gpsimd
                                    if h == 0 and j in (1, 3, 5, 6)
                                    else nc.vector
                                )
                                eng.tensor_tensor(
                                    em[:], em[:], nm_j[j][:], MULT,
                                )
                                if _dbg and rep == 0 and (p, j, h) == (0, 0, 0):
                                    nc.sync.dma_start(em_d.ap(), em[:])

                        O_cur = {}
                        Oc_cur = {}

                        def emit_attn_burst(p, h, c):
                            # closed 8-matmul accumulation burst for pair p
                            if (p, h) not in O_cur:
                                O_cur[(p, h)] = psO.tile(
                                    [DKP, S], F32, tag="O",
                                    name=f"O_{rep}_{p}_{h}",
                                )
                            O = O_cur[(p, h)]
                            head = 2 * p + h
                            chalf = head // 8
                            jseq = [NT - 1] + list(range(NT - 1))
                            for i, j in enumerate(jseq):
                                rhs = (
                                    nm_j[j][:, c * 512 : (c + 1) * 512]
                                    if (_peonly or _expstage) else
                                    em_tiles[(p, j, h)][:, c * 512 : (c + 1) * 512]
                                )
                                mm = pe_mm(
                                    O[:, c * 512 : (c + 1) * 512],
                                    Vg[:, j, head * DKP : (head + 1) * DKP],
                                    rhs,
                                    start=(i == 0), stop=(i == NT - 1),
                                )
                                if i == 0:
                                    # the tile framework does not track the
                                    # strided Vg writes; sync the burst on
                                    # the last v-chunk copy of its c-half
                                    dep = _vg_copy.get((NT - 1, chalf))
                                    if dep is not None:
                                        add_dep_helper(
                                            mm.ins, dep, sync=True,
                                            reason="Vg strided-write race",
                                        )

                        def emit_norm_a(p, h):
                            if _peonly or _expstage or _maskstage:
                                nc.vector.tensor_copy(
                                    oT_t[p][0:DKP, 0:16],
                                    O_cur.pop((p, h))[:, 0:16],
                                )
                                return None
                            # reciprocal of both heads' softmax sums into one
                            # [2, S] tile; one DRAM-bounce broadcast to
                            # [2*DK, S] per pair (HWDGE instruction count)
                            O = O_cur[(p, h)]
                            if p == NPAIR - 1:
                                R7 = p_r.tile(
                                    [1, S], F32, tag="R", name=f"R7_{rep}_{h}"
                                )
                                nc.vector.reciprocal(R7[:], O[DK : DK + 1, :])
                                Oc = p_oc.tile(
                                    [DK, S], F16, tag="Oc",
                                    name=f"Oc_{rep}_{p}_{h}",
                                )
                                nc.vector.tensor_copy(Oc[:], O[0:DK, :])
                                O_cur.pop((p, h))
                                Oc_cur[(p, h)] = Oc
                                return R7
                            if h == 0:
                                R2 = p_r.tile(
                                    [2 * 32, S], F16, tag="R",
                                    name=f"R_{rep}_{p}",
                                )
                                emit_norm_a.r2 = R2
                            else:
                                R2 = emit_norm_a.r2
                            with nc.allow_low_precision(
                                reason="1/softmax-sum bounce in fp16; 5e-4 "
                                "rel err on normalized weights is fine"
                            ):
                                nc.vector.reciprocal(
                                    R2[32 * h : 32 * h + 1, :],
                                    O[DK : DK + 1, :],
                                )
                            Oc = p_oc.tile(
                                [DK, S], F16, tag="Oc",
                                name=f"Oc_{rep}_{p}_{h}",
                            )
                            nc.vector.tensor_copy(Oc[:], O[0:DK, :])
                            O_cur.pop((p, h))
                            Oc_cur[(p, h)] = Oc
                            if h == 0:
                                return None
                            Rd = p_dram.tile(
                                [2, S], F16, tag="Rd", name=f"Rd_{rep}_{p}"
                            )
                            nc.scalar.dma_start(
                                Rd[:],
                                R2[:].rearrange(
                                    "(a b) f -> a b f", b=32
                                )[:, 0:1, :],
                            )
                            Rbs = []
                            for hh in range(2):
                                Rbh = p_rb.tile(
                                    [DK, S], F16, tag="Rb",
                                    name=f"Rb_{rep}_{p}_{hh}",
                                )
                                nc.scalar.dma_start(
                                    Rbh[:],
                                    Rd[hh : hh + 1, :].broadcast_to((DK, S)),
                                )
                                Rbs.append(Rbh)
                            return Rbs

                        def emit_norm_b(p, h, Rb):
                            if _peonly or _expstage or _maskstage:
                                return
                            O = Oc_cur.pop((p, h))
                            Rbh = Rb[h] if isinstance(Rb, list) else Rb
                            nc.vector.tensor_tensor(
                                oT_t[p][h * DK : (h + 1) * DK, :],
                                O[0:DK, :], Rbh[:], MULT,
                            )

                        # ---- head: pair-0 q/k projections ----
                        for c in range(2):
                            emit_proj_chunk("q", 0, c)
                        for c in range(2):
                            emit_proj_chunk("k", 0, c)

                        if _dbg:
                            nc.sync.dma_start(qt0_d.ap(), QT_p[0][:])
                            nc.sync.dma_start(kt0_d.ap(), KT_p[0][:])
                        # ---- pair windows ----
                        norm_rb = {}
                        for p in range(NPAIR):
                            if p == NPAIR - 1:
                                # Wq is dead; reuse its SBUF for Wo.
                                nc.scalar.dma_start(
                                    Wf["q"][:], w_t["o"][:]
                                )
                            # absorber queue for this window
                            absq = []
                            if p > 0:
                                for h, c in ((0, 0), (0, 1), (1, 0), (1, 1)):
                                    absq.append(("burst", p - 1, h, c))
                            if p < NPAIR - 1:
                                for pr in ("q", "k"):
                                    for c in range(2):
                                        absq.append(("chunk", pr, p + 1, c))
                            for t, c in _VMAP[p]:
                                absq.append(("chunk", "v", t, c))
                            # interleave: spread absorbers evenly across the
                            # 8 quartet slots (emit after each quartet until
                            # the backlog matches the remaining slots)
                            total = len(absq)

                            def pop_abs():
                                op = absq.pop(0)
                                if op[0] == "burst":
                                    _, bp, bh, bc = op
                                    emit_attn_burst(bp, bh, bc)
                                    if bc == 1:
                                        rbv = emit_norm_a(bp, bh)
                                        if rbv is not None:
                                            norm_rb[bp] = rbv
                                else:
                                    _, pr, t, c = op
                                    emit_proj_chunk(pr, t, c)

                            for j in range(NT):
                                emit_logits_exp(p, j)
                                want_done = (total * (j + 1) + NT - 1) // NT
                                while absq and total - len(absq) < want_done:
                                    pop_abs()
                                if j >= 2:
                                    emit_mask(p, j - 2)
                            while absq:
                                pop_abs()
                            emit_mask(p, NT - 2)
                            emit_mask(p, NT - 1)
                            if p > 0 and not (_peonly or _expstage or _maskstage):
                                rbv = norm_rb.pop(p - 1)
                                for hh in range(2):
                                    emit_norm_b(p - 1, hh, rbv)


                        if _dbg:
                            nc.sync.dma_start(ot0_d.ap(), oT_t[0][:])
                            nc.sync.dma_start(vg0_d.ap(), Vg[:, 0, :])
                        # ---- tail: attnV(7), PE-broadcast norm, out-proj ----
                        R_hist = {}
                        for h, c in ((0, 0), (0, 1), (1, 0), (1, 1)):
                            emit_attn_burst(NPAIR - 1, h, c)
                            if c == 1:
                                R_hist[h] = emit_norm_a(NPAIR - 1, h)
                        psL.release()
                        with tc.tile_pool(name="psC", bufs=2, space="PSUM") as psC:
                            p7 = NPAIR - 1
                            if _peonly or _expstage or _maskstage:
                                nc.gpsimd.memset(oT_t[0][:], 0.5)
                                nc.gpsimd.memset(oT_t[1][:, 16:S], 0.5)
                                for jj in range(1, NT):
                                    nc.gpsimd.memset(oT_t[jj][:], 0.5)
                            for h in range(2 * (0 if (_peonly or _expstage or _maskstage) else 1)):
                                Rp = psC.tile(
                                    [DK, S], F32, tag="F", name=f"Rp_{rep}_{h}"
                                )
                                for c in range(2):
                                    pe_mm(
                                        Rp[:, c * 512 : (c + 1) * 512],
                                        ones1[:],
                                        R_hist[h][:, c * 512 : (c + 1) * 512],
                                        start=True, stop=True,
                                    )
                                Rs = p_rb.tile(
                                    [DK, S], F32, tag="Rb", name=f"Rs_{rep}_{h}"
                                )
                                nc.vector.tensor_copy(Rs[:], Rp[:])
                                emit_norm_b(p7, h, Rs)
                            for t in range(NT):
                                F = psC.tile(
                                    [P, S], F32, tag="F", name=f"F_{rep}_{t}"
                                )
                                for j in range(NT):
                                    for c in range(2):
                                        pe_mm(
                                            F[:, c * 512 : (c + 1) * 512],
                                            oT_t[j][:, t * P : (t + 1) * P],
                                            Wf["q"][:, j, c * 512 : (c + 1) * 512],
                                            start=(j == 0), stop=(j == NT - 1),
                                        )
                                ot = p_out.tile(
                                    [P, S], F16, tag="ot", name=f"ot_{rep}_{t}"
                                )
                                nc.vector.tensor_add(ot[:], F[:], bo_sb[:])
                                nc.sync.dma_start(out_t[:, t, :], ot[:])

    _cached_nc[repeat] = nc
    return nc


# ---------------------------------------------------------------------------
# Entry point
# ---------------------------------------------------------------------------
def make_in_maps(x, attn_mask, Wq, bq, Wk, bk, Wv, bv, Wo, bo):
    ndt = np.float16
    Wk64 = np.asarray(Wk, np.float64)
    bq64 = np.asarray(bq, np.float64)
    # per-head exp bias direction: wb[:, h] = (Wk_h @ bq_h) / 8
    WB = np.stack(
        [
            Wk64[:, h * DK : (h + 1) * DK] @ bq64[h * DK : (h + 1) * DK] / 8.0
            for h in range(H)
        ],
        axis=1,
    )  # [E, H]
    bo_eff = (
        np.asarray(bv, np.float64) @ np.asarray(Wo, np.float64)
        + np.asarray(bo, np.float64)
    ).astype(np.float32)
    wqc = np.asarray(Wq, np.float32).astype(ndt)
    wkc = np.asarray(Wk, np.float32).astype(ndt)
    wvc = np.asarray(Wv, np.float32).astype(ndt)
    woc = np.asarray(Wo, np.float32).astype(ndt)
    in_maps = []
    for n in range(N):
        notm_t = np.ascontiguousarray(
            (1.0 - np.asarray(attn_mask[n], np.float32)).T
        ).astype(np.float16)
        B = np.asarray(x[n], np.float64) @ WB  # [S, H]
        btc = np.ascontiguousarray(
            B.reshape(NT, P, H).transpose(1, 0, 2).reshape(P, NT * H)
        ).astype(np.float32)
        in_maps.append(
            {
                "x_t": np.ascontiguousarray(np.asarray(x[n], np.float32).T).astype(ndt),
                "nm_t": notm_t,
                "wq": wqc, "wk": wkc, "wv": wvc, "wo": woc,
                "bt": btc, "bo_eff": bo_eff,
            }
        )
    return in_maps


def kernel(x, attn_mask, Wq, bq, Wk, bk, Wv, bv, Wo, bo, **_):
    nc = _build()
    in_maps = make_in_maps(x, attn_mask, Wq, bq, Wk, bk, Wv, bv, Wo, bo)
    res = run_bass_kernel_spmd(nc, in_maps, list(range(N)))
    outs = np.stack([np.asarray(res.results[n]["out"]) for n in range(N)], axis=0)
    return outs.astype(np.float32)


# revision 37
# speedup vs baseline: 1.1894x; 1.0793x over previous
"""Trainium2 Bass kernel for batched multi-head attention.

Problem: N=8, S=1024, E=1024, H=16, DK=64 MultiHeadAttention with a boolean
attention mask, fp32 reference.

Strategy: pure batch data-parallelism -- one batch element per NeuronCore
(8 cores), weights replicated, no collectives.  Per core everything is
computed in a transposed layout so no on-chip transposes are needed:

  xT [E, S] (host-transposed)  --Wq/Wk-->  QT, KT [E, S]  (no bias: the
      q-side bias and const cancel in softmax; the k-side bias folds into
      the exp bias B[s,h] = x[s]@(Wk_h bq_h)/8, precomputed on host)
  xT                           --Wv---->   V [S, E] head-major w/ ones col
  logitsT[k, q] = KT_h^T-slices @ QT_h    (K=64 row-paired matmuls on PE
      subarray tiles (0,0)/(64,0) -- two heads' matmuls run concurrently)
  Em = exp(logitsT/8 + B) * notm          (ACT exp w/ per-partition bias,
                                           DVE mask multiply; Em -> SBUF
                                           fp16 ring, 2 pairs deep)
  O_h[d|sum, q] = V_aug_h^T @ Em_h        (CLOSED 8-matmul accumulation
      bursts, delayed one pair: attnV for pair p runs during pair p+1's
      logits window.  Keeping accumulation groups closed around the 64-row
      logits matmuls avoids a ~10x PE penalty when >=2 PSUM groups are
      open across tile-size switches -- measured on HW.)
  oT[e', q] = O_h[0:64] * (1/sums)        (DVE; 1/sums broadcast across
      partitions via DRAM bounce; pair 7 uses a PE broadcast instead)
  out[q, e] = oT^T-slices @ Wo + bo_eff   (fp16 out, host casts to fp32)
"""

import numpy as np
from contextlib import ExitStack

import concourse.bass as bass
import concourse.mybir as mybir
import concourse.tile as tile
from concourse.tile_rust import add_dep_helper
from concourse.vector_clock import ScopedClock
from concourse.bass_utils import run_bass_kernel_spmd

F32 = mybir.dt.float32
F16 = mybir.dt.float16
Exp = mybir.ActivationFunctionType.Exp
MULT = mybir.AluOpType.mult

N, S, E, H, DK = 8, 1024, 1024, 16, 64
P = 128
NT = E // P
NPAIR = H // 2
DKP = DK + 1  # head slot width in V_aug (64 values + ones column)

MM_DT = F16


# ---------------------------------------------------------------------------
# Workaround: this walrus build supports at most ONE semaphore wait per
# instruction.  Split instructions carrying more waits into NOP(wait) chains
# on the same engine, and do the same for the TileContext tail drain.
# ---------------------------------------------------------------------------
_MAXW = 1
_orig_lower = tile.TileContext._lower_ordered_insts
_tilefix_installed = False


def _split_waits(ordered):
    for _bb, insts in ordered.items():
        out = []
        for inst in insts:
            si = inst.sync_info
            if si is not None and len(si.on_wait) > _MAXW:
                waits = list(si.on_wait)
                keep, extra = waits[:_MAXW], waits[_MAXW:]
                for i in range(0, len(extra), _MAXW):
                    out.append(
                        mybir.InstNoOp(
                            name=f"{inst.name}-ws{i}",
                            engine=inst.engine,
                            bass_nofuse=True,
                            sync_info=mybir.SyncInfo(
                                on_wait=extra[i : i + _MAXW], on_update=[]
                            ),
                        )
                    )
                inst.sync_info = mybir.SyncInfo(
                    on_wait=keep, on_update=list(si.on_update)
                )
            out.append(inst)
        insts[:] = out


def _patched_lower(self, ordered):
    _split_waits(ordered)
    return _orig_lower(self, ordered)


def _patched_drain_and_barrier(self, tick_clock, wait_clock):
    nc = self.nc
    drain_inst = nc.sync.drain()
    wait_clock.add_sem_waits(
        drain_inst.ins, ScopedClock({None: tick_clock.global_clock})
    )
    si = drain_inst.ins.sync_info
    waits = list(si.on_wait) if si is not None else []
    if len(waits) > _MAXW:
        drain_inst.ins.sync_info = mybir.SyncInfo(on_wait=[], on_update=[])
        for i in range(0, len(waits), _MAXW):
            nop = nc.sync.nop(nofuse=True)
            nop.ins.sync_info = mybir.SyncInfo(
                on_wait=waits[i : i + _MAXW], on_update=[]
            )
    nc.all_engine_barrier()
    popped = nc._tile_sem_poison_stack.pop()
    assert popped is self._sem_poison
    nc.clear_and_free_semaphores(list(self.sems.allocated().values()))
    nc.all_engine_barrier()


def _install_tilefix():
    global _tilefix_installed
    if not _tilefix_installed:
        tile.TileContext._lower_ordered_insts = _patched_lower
        tile.TileContext._drain_and_barrier = _patched_drain_and_barrier
        _tilefix_installed = True


# ---------------------------------------------------------------------------
# Kernel build
# ---------------------------------------------------------------------------
_cached_nc = {}

# v-proj chunk placement: window -> list of (t, c) emitted as absorbers
_VMAP = {
    0: [(t, 0) for t in range(NT)],
    1: [(0, 1), (1, 1)],
    2: [(2, 1), (3, 1)],
    3: [(4, 1), (5, 1)],
    4: [(6, 1), (7, 1)],
    5: [],
    6: [],
    7: [],
}


def _build(repeat=1):
    if repeat in _cached_nc:
        return _cached_nc[repeat]
    _install_tilefix()

    nc = bass.Bass("TRN2", num_devices=N)

    x_t = nc.declare_dram_parameter("x_t", [E, S], MM_DT, isOutput=False)
    nm_t = nc.declare_dram_parameter("nm_t", [S, S], F16, isOutput=False)
    wq = nc.declare_dram_parameter("wq", [E, E], MM_DT, isOutput=False)
    wk = nc.declare_dram_parameter("wk", [E, E], MM_DT, isOutput=False)
    wv = nc.declare_dram_parameter("wv", [E, E], MM_DT, isOutput=False)
    wo = nc.declare_dram_parameter("wo", [E, E], MM_DT, isOutput=False)
    bt = nc.declare_dram_parameter("bt", [P, NT * H], F32, isOutput=False)
    bo_eff = nc.declare_dram_parameter("bo_eff", [E], F32, isOutput=False)
    out = nc.declare_dram_parameter("out", [S, E], F16, isOutput=True)
    import os as _os
    _dbg = _os.environ.get("KDEBUG") == "1"
    _peonly = _os.environ.get("KPEONLY") == "1"
    _exponly = False
    _expstage = _os.environ.get("KSTAGE") == "exp"
    _maskstage = _os.environ.get("KSTAGE") == "mask"
    if _dbg:
        qt0_d = nc.declare_dram_parameter("qt0_d", [P, S], MM_DT, isOutput=True)
        kt0_d = nc.declare_dram_parameter("kt0_d", [P, S], MM_DT, isOutput=True)
        em_d = nc.declare_dram_parameter("em_d", [P, S], F16, isOutput=True)
        o00_d = nc.declare_dram_parameter("o00_d", [DKP, S], F32, isOutput=True)
        rb00_d = nc.declare_dram_parameter("rb00_d", [DK, S], F32, isOutput=True)
        ot0_d = nc.declare_dram_parameter("ot0_d", [P, S], MM_DT, isOutput=True)
        vg0_d = nc.declare_dram_parameter("vg0_d", [P, H * DKP], F16, isOutput=True)

    def tiled(ap):
        return ap.rearrange("(t p) f -> p t f", p=P)

    x_tt = tiled(x_t.ap())
    nm_tt = tiled(nm_t.ap())
    w_t = {
        "q": tiled(wq.ap()),
        "k": tiled(wk.ap()),
        "v": tiled(wv.ap()),
        "o": tiled(wo.ap()),
    }
    bt_t = bt.ap().rearrange("p (t h) -> p t h", h=H)
    out_t = tiled(out.ap())

    with tile.TileContext(nc) as tc, ExitStack() as ctx:
        p_pers = ctx.enter_context(tc.tile_pool(name="pers", bufs=1))
        p_qk = ctx.enter_context(tc.tile_pool(name="qk", bufs=3))
        QT_p = {}
        KT_p = {}
        oT_t = [p_pers.tile([P, S], MM_DT, name=f"oT{j}") for j in range(NT)]
        nm_all = p_pers.tile([P, NT, S], F16, name="nm_all")
        nm_j = [nm_all[:, j, :] for j in range(NT)]
        Vg = p_pers.tile([P, NT, H * DKP], F16)
        BT_sb = p_pers.tile([P, NT, H], F32)
        ones1 = p_pers.tile([1, DK], F32)
        bo_sb = p_pers.tile([P, S], F32)

        for rep in range(repeat):
            with tc.tile_pool(name="em", bufs=(22 if _dbg else 24)) as p_em, \
                 tc.tile_pool(name="rr", bufs=2) as p_r, \
                 tc.tile_pool(name="rb", bufs=3) as p_rb, \
                 tc.tile_pool(name="oc", bufs=3) as p_oc, \
                 tc.tile_pool(name="po", bufs=2) as p_out, \
                 tc.tile_pool(name="dram", bufs=8, space="DRAM") as p_dram:

                QT_p.clear()
                KT_p.clear()
                nc.gpsimd.memset(Vg[:, :, DK::DKP], 1.0)
                nc.gpsimd.memset(ones1[:], 1.0)

                with tc.tile_pool(name="w", bufs=1) as p_w:
                    xT = p_w.tile([P, NT, S], MM_DT, name=f"xT_{rep}")
                    Wf = {
                        pr: p_w.tile([P, NT, S], MM_DT, name=f"Wf_{rep}_{pr}")
                        for pr in ("q", "k", "v")
                    }
                    # DMA priority order for cold start (batched: HWDGE
                    # costs ~630ns of serialized overhead PER instruction):
                    # SP queue: x + W t0-slices unblock pair-0 projections,
                    # then the Wq/Wk remainders.  ACT queue: mask, V, consts.
                    nc.sync.dma_start(xT[:], x_tt[:])
                    nc.sync.dma_start(Wf["q"][:, :, 0:P], w_t["q"][:, :, 0:P])
                    nc.sync.dma_start(Wf["k"][:, :, 0:P], w_t["k"][:, :, 0:P])
                    nc.scalar.dma_start(BT_sb[:], bt_t[:])
                    nc.scalar.dma_start(nm_all[:], nm_tt[:])
                    nc.sync.dma_start(Wf["q"][:, :, P:S], w_t["q"][:, :, P:S])
                    nc.sync.dma_start(Wf["k"][:, :, P:S], w_t["k"][:, :, P:S])
                    nc.scalar.dma_start(
                        Wf["v"][:, :, 0:512], w_t["v"][:, :, 0:512]
                    )
                    nc.scalar.dma_start(
                        Wf["v"][:, :, 512:1024], w_t["v"][:, :, 512:1024]
                    )
                    nc.scalar.dma_start(
                        bo_sb[:],
                        bo_eff.ap().rearrange("(o e) -> o e", o=1).broadcast_to((P, S)),
                    )

                    with tc.tile_pool(name="psO", bufs=1, space="PSUM") as psO:
                        psL = tc.alloc_tile_pool(name="psL", bufs=2, space="PSUM")

                        _pe_prev = [None]
                        _vg_copy = {}

                        def pe_mm(*args, chain=True, **kw):
                            # pin PE program order to emission order: the
                            # list scheduler otherwise hoists matmuls by
                            # readiness, splitting accumulation bursts
                            # around 64-row tiled matmuls (a ~10x HW
                            # penalty when >=2 PSUM groups are open across
                            # a tile-size switch) and racing ahead of Vg
                            # writers whose strided copies it won't track.
                            h = nc.tensor.matmul(*args, **kw)
                            if chain:
                                if _pe_prev[0] is not None:
                                    add_dep_helper(
                                        h.ins, _pe_prev[0], sync=False,
                                        reason="pe absorber-unit order",
                                    )
                                _pe_prev[0] = h.ins
                            return h

                        def emit_proj_chunk(proj, t, c):
                            acc = psL.tile(
                                [P, 512], F32, tag="A", bufs=2,
                                name=f"a2_{rep}_{proj}_{t}_{c}",
                            )
                            for j in range(NT):
                                wt = Wf[proj][:, j, :]
                                if proj == "v":
                                    lhsT = xT[:, j, t * P : (t + 1) * P]
                                    rhs = wt[:, c * 512 : (c + 1) * 512]
                                else:
                                    lhsT = wt[:, t * P : (t + 1) * P]
                                    rhs = xT[:, j, c * 512 : (c + 1) * 512]
                                pe_mm(
                                    acc[:], lhsT, rhs,
                                    start=(j == 0), stop=(j == NT - 1),
                                )
                            if proj == "v":
                                dst = Vg[
                                    :, t, c * 8 * DKP : (c + 1) * 8 * DKP
                                ].rearrange("p (h d) -> p h d", d=DKP)[:, :, 0:DK]
                                src = acc[:].rearrange("p (h d) -> p h d", d=DK)
                                cp = nc.vector.tensor_copy(dst, src)
                                _vg_copy[(t, c)] = cp.ins
                            else:
                                dst, tg = (
                                    (QT_p, "QT") if proj == "q" else (KT_p, "KT")
                                )
                                if t not in dst:
                                    dst[t] = p_qk.tile(
                                        [P, S], MM_DT, tag=tg,
                                        name=f"{tg}_{rep}_{t}",
                                    )
                                nc.vector.tensor_copy(
                                    dst[t][:, c * 512 : (c + 1) * 512], acc[:]
                                )

                        em_tiles = {}

                        def emit_logits_exp(p, j):
                            # four 1-bank L units per quartet; exps split per
                            # (h, c-half) so psL units release every ~570ns
                            # instead of per 2us exp pair (shrinks the
                            # PE<->ACT round-trip lockstep)
                            L = {}
                            for h in range(2):
                                L[h] = psL.tile(
                                    [P, S], F32, tag="L",
                                    name=f"L_{rep}_{p}_{j}_{h}",
                                )
                            for c in range(2):
                                for h in range(2):
                                    r0, r1 = h * DK, (h + 1) * DK
                                    pe_mm(
                                        L[h][:, c * 512 : (c + 1) * 512],
                                        KT_p[p][r0:r1, j * P : (j + 1) * P],
                                        QT_p[p][r0:r1, c * 512 : (c + 1) * 512],
                                        start=True, stop=True,
                                        tile_position=(r0, 0),
                                        chain=False,
                                    )
                            if _peonly:
                                for h in range(2):
                                    nc.vector.tensor_copy(
                                        oT_t[0][0:1, j * 16 : j * 16 + 16],
                                        L[h][0:1, 0:16],
                                    )
                                return
                            ems = {}
                            for h in range(2):
                                ems[h] = p_em.tile(
                                    [P, S], F16,
                                    tag=("Eme" if _expstage else "Em"),
                                    name=f"Em_{rep}_{p}_{j}_{h}",
                                    bufs=(3 if _expstage else None),
                                )
                                em_tiles[(p, j, h)] = ems[h]
                            import os as _os
                            for h in range(2):
                                _bias = (
                                    0.0 if _os.environ.get("KBIAS") == "0"
                                    else BT_sb[:, j, 2 * p + h : 2 * p + h + 1]
                                )
                                nc.scalar.activation(
                                    ems[h][:], L[h][:], Exp, scale=0.125,
                                    bias=_bias,
                                )
                            if _expstage:
                                nc.vector.tensor_copy(
                                    oT_t[1][0:1, 0:16], ems[0][0:1, 0:16],
                                )

                        def emit_mask(p, j):
                            if _expstage:
                                return
                            for h in range(2):
                                em = em_tiles[(p, j, h)]
                                eng = (
                                    nc.gpsimd
                                    if h == 0 and j in (1, 3, 5, 6)
                                    else nc.vector
                                )
                                eng.tensor_tensor(
                                    em[:], em[:], nm_j[j][:], MULT,
                                )
                                if _dbg and rep == 0 and (p, j, h) == (0, 0, 0):
                                    nc.sync.dma_start(em_d.ap(), em[:])

                        O_cur = {}
                        Oc_cur = {}

                        def emit_attn_burst(p, h, c):
                            # closed 8-matmul accumulation burst for pair p
                            if (p, h) not in O_cur:
                                O_cur[(p, h)] = psO.tile(
                                    [DKP, S], F32, tag="O",
                                    name=f"O_{rep}_{p}_{h}",
                                )
                            O = O_cur[(p, h)]
                            head = 2 * p + h
                            chalf = head // 8
                            jseq = [NT - 1] + list(range(NT - 1))
                            for i, j in enumerate(jseq):
                                rhs = (
                                    nm_j[j][:, c * 512 : (c + 1) * 512]
                                    if (_peonly or _expstage) else
                                    em_tiles[(p, j, h)][:, c * 512 : (c + 1) * 512]
                                )
                                mm = pe_mm(
                                    O[:, c * 512 : (c + 1) * 512],
                                    Vg[:, j, head * DKP : (head + 1) * DKP],
                                    rhs,
                                    start=(i == 0), stop=(i == NT - 1),
                                )
                                if i == 0:
                                    # the tile framework does not track the
                                    # strided Vg writes; sync the burst on
                                    # the last v-chunk copy of its c-half
                                    dep = _vg_copy.get((NT - 1, chalf))
                                    if dep is not None:
                                        add_dep_helper(
                                            mm.ins, dep, sync=True,
                                            reason="Vg strided-write race",
                                        )

                        def emit_norm_a(p, h):
                            if _peonly or _expstage or _maskstage:
                                nc.vector.tensor_copy(
                                    oT_t[p][0:DKP, 0:16],
                                    O_cur.pop((p, h))[:, 0:16],
                                )
                                return None
                            # reciprocal of both heads' softmax sums into one
                            # [2, S] tile; one DRAM-bounce broadcast to
                            # [2*DK, S] per pair (HWDGE instruction count)
                            O = O_cur[(p, h)]
                            if p == NPAIR - 1:
                                R7 = p_r.tile(
                                    [1, S], F32, tag="R", name=f"R7_{rep}_{h}"
                                )
                                nc.vector.reciprocal(R7[:], O[DK : DK + 1, :])
                                Oc = p_oc.tile(
                                    [DK, S], F16, tag="Oc",
                                    name=f"Oc_{rep}_{p}_{h}",
                                )
                                nc.vector.tensor_copy(Oc[:], O[0:DK, :])
                                O_cur.pop((p, h))
                                Oc_cur[(p, h)] = Oc
                                return R7
                            if h == 0:
                                R2 = p_r.tile(
                                    [2 * 32, S], F16, tag="R",
                                    name=f"R_{rep}_{p}",
                                )
                                emit_norm_a.r2 = R2
                            else:
                                R2 = emit_norm_a.r2
                            with nc.allow_low_precision(
                                reason="1/softmax-sum bounce in fp16; 5e-4 "
                                "rel err on normalized weights is fine"
                            ):
                                nc.vector.reciprocal(
                                    R2[32 * h : 32 * h + 1, :],
                                    O[DK : DK + 1, :],
                                )
                            Oc = p_oc.tile(
                                [DK, S], F16, tag="Oc",
                                name=f"Oc_{rep}_{p}_{h}",
                            )
                            nc.vector.tensor_copy(Oc[:], O[0:DK, :])
                            O_cur.pop((p, h))
                            Oc_cur[(p, h)] = Oc
                            if h == 0:
                                return None
                            Rd = p_dram.tile(
                                [2, S], F16, tag="Rd", name=f"Rd_{rep}_{p}"
                            )
                            nc.scalar.dma_start(
                                Rd[:],
                                R2[:].rearrange(
                                    "(a b) f -> a b f", b=32
                                )[:, 0:1, :],
                            )
                            Rbs = []
                            for hh in range(2):
                                Rbh = p_rb.tile(
                                    [DK, S], F16, tag="Rb",
                                    name=f"Rb_{rep}_{p}_{hh}",
                                )
                                nc.scalar.dma_start(
                                    Rbh[:],
                                    Rd[hh : hh + 1, :].broadcast_to((DK, S)),
                                )
                                Rbs.append(Rbh)
                            return Rbs

                        def emit_norm_b(p, h, Rb):
                            if _peonly or _expstage or _maskstage:
                                return
                            O = Oc_cur.pop((p, h))
                            Rbh = Rb[h] if isinstance(Rb, list) else Rb
                            nc.vector.tensor_tensor(
                                oT_t[p][h * DK : (h + 1) * DK, :],
                                O[0:DK, :], Rbh[:], MULT,
                            )

                        # ---- head: pair-0 q/k projections ----
                        for c in range(2):
                            emit_proj_chunk("q", 0, c)
                        for c in range(2):
                            emit_proj_chunk("k", 0, c)

                        if _dbg:
                            nc.sync.dma_start(qt0_d.ap(), QT_p[0][:])
                            nc.sync.dma_start(kt0_d.ap(), KT_p[0][:])
                        # ---- pair windows ----
                        norm_rb = {}
                        for p in range(NPAIR):
                            if p == NPAIR - 1:
                                # Wq is dead; reuse its SBUF for Wo.
                                nc.scalar.dma_start(
                                    Wf["q"][:], w_t["o"][:]
                                )
                            # absorber queue for this window
                            absq = []
                            if p > 0:
                                absq += [("burst", p - 1, 0, 0),
                                         ("burst", p - 1, 0, 1)]
                            if p < NPAIR - 1:
                                for c in range(2):
                                    absq.append(("chunk", "q", p + 1, c))
                            if p > 0:
                                absq += [("burst", p - 1, 1, 0),
                                         ("burst", p - 1, 1, 1)]
                            if p < NPAIR - 1:
                                for c in range(2):
                                    absq.append(("chunk", "k", p + 1, c))
                            for t, c in _VMAP[p]:
                                absq.append(("chunk", "v", t, c))
                            # interleave: spread absorbers evenly across the
                            # 8 quartet slots (emit after each quartet until
                            # the backlog matches the remaining slots)
                            total = len(absq)

                            def pop_abs():
                                op = absq.pop(0)
                                if op[0] == "burst":
                                    _, bp, bh, bc = op
                                    emit_attn_burst(bp, bh, bc)
                                    if bc == 1:
                                        rbv = emit_norm_a(bp, bh)
                                        if rbv is not None:
                                            norm_rb[bp] = rbv
                                else:
                                    _, pr, t, c = op
                                    emit_proj_chunk(pr, t, c)

                            for j in range(NT):
                                emit_logits_exp(p, j)
                                want_done = (total * (j + 1) + NT - 1) // NT
                                while absq and total - len(absq) < want_done:
                                    pop_abs()
                                if j >= 2:
                                    emit_mask(p, j - 2)
                            while absq:
                                pop_abs()
                            emit_mask(p, NT - 2)
                            emit_mask(p, NT - 1)
                            if p > 0 and not (_peonly or _expstage or _maskstage):
                                rbv = norm_rb.pop(p - 1)
                                for hh in range(2):
                                    emit_norm_b(p - 1, hh, rbv)


                        if _dbg:
                            nc.sync.dma_start(ot0_d.ap(), oT_t[0][:])
                            nc.sync.dma_start(vg0_d.ap(), Vg[:, 0, :])
                        # ---- tail: attnV(7), PE-broadcast norm, out-proj ----
                        R_hist = {}
                        for h, c in ((0, 0), (0, 1), (1, 0), (1, 1)):
                            emit_attn_burst(NPAIR - 1, h, c)
                            if c == 1:
                                R_hist[h] = emit_norm_a(NPAIR - 1, h)
                        psL.release()
                        with tc.tile_pool(name="psC", bufs=2, space="PSUM") as psC:
                            p7 = NPAIR - 1
                            if _peonly or _expstage or _maskstage:
                                nc.gpsimd.memset(oT_t[0][:], 0.5)
                                nc.gpsimd.memset(oT_t[1][:, 16:S], 0.5)
                                for jj in range(1, NT):
                                    nc.gpsimd.memset(oT_t[jj][:], 0.5)
                            for h in range(2 * (0 if (_peonly or _expstage or _maskstage) else 1)):
                                Rp = psC.tile(
                                    [DK, S], F32, tag="F", name=f"Rp_{rep}_{h}"
                                )
                                for c in range(2):
                                    pe_mm(
                                        Rp[:, c * 512 : (c + 1) * 512],
                                        ones1[:],
                                        R_hist[h][:, c * 512 : (c + 1) * 512],
                                        start=True, stop=True,
                                    )
                                Rs = p_rb.tile(
                                    [DK, S], F32, tag="Rb", name=f"Rs_{rep}_{h}"
                                )
                                nc.vector.tensor_copy(Rs[:], Rp[:])
                                emit_norm_b(p7, h, Rs)
                            for t in range(NT):
                                F = psC.tile(
                                    [P, S], F32, tag="F", name=f"F_{rep}_{t}"
                                )
                                for j in range(NT):
                                    for c in range(2):
                                        pe_mm(
                                            F[:, c * 512 : (c + 1) * 512],
                                            oT_t[j][:, t * P : (t + 1) * P],
                                            Wf["q"][:, j, c * 512 : (c + 1) * 512],
                                            start=(j == 0), stop=(j == NT - 1),
                                        )
                                ot = p_out.tile(
                                    [P, S], F16, tag="ot", name=f"ot_{rep}_{t}"
                                )
                                nc.vector.tensor_add(ot[:], F[:], bo_sb[:])
                                nc.sync.dma_start(out_t[:, t, :], ot[:])

    _cached_nc[repeat] = nc
    return nc


# ---------------------------------------------------------------------------
# Entry point
# ---------------------------------------------------------------------------
def make_in_maps(x, attn_mask, Wq, bq, Wk, bk, Wv, bv, Wo, bo):
    ndt = np.float16
    Wk64 = np.asarray(Wk, np.float64)
    bq64 = np.asarray(bq, np.float64)
    # per-head exp bias direction: wb[:, h] = (Wk_h @ bq_h) / 8
    WB = np.stack(
        [
            Wk64[:, h * DK : (h + 1) * DK] @ bq64[h * DK : (h + 1) * DK] / 8.0
            for h in range(H)
        ],
        axis=1,
    )  # [E, H]
    bo_eff = (
        np.asarray(bv, np.float64) @ np.asarray(Wo, np.float64)
        + np.asarray(bo, np.float64)
    ).astype(np.float32)
    wqc = np.asarray(Wq, np.float32).astype(ndt)
    wkc = np.asarray(Wk, np.float32).astype(ndt)
    wvc = np.asarray(Wv, np.float32).astype(ndt)
    woc = np.asarray(Wo, np.float32).astype(ndt)
    in_maps = []
    for n in range(N):
        notm_t = np.ascontiguousarray(
            (1.0 - np.asarray(attn_mask[n], np.float32)).T
        ).astype(np.float16)
        B = np.asarray(x[n], np.float64) @ WB  # [S, H]
        btc = np.ascontiguousarray(
            B.reshape(NT, P, H).transpose(1, 0, 2).reshape(P, NT * H)
        ).astype(np.float32)
        in_maps.append(
            {
                "x_t": np.ascontiguousarray(np.asarray(x[n], np.float32).T).astype(ndt),
                "nm_t": notm_t,
                "wq": wqc, "wk": wkc, "wv": wvc, "wo": woc,
                "bt": btc, "bo_eff": bo_eff,
            }
        )
    return in_maps


def kernel(x, attn_mask, Wq, bq, Wk, bk, Wv, bv, Wo, bo, **_):
    nc = _build()
    in_maps = make_in_maps(x, attn_mask, Wq, bq, Wk, bk, Wv, bv, Wo, bo)
    res = run_bass_kernel_spmd(nc, in_maps, list(range(N)))
    outs = np.stack([np.asarray(res.results[n]["out"]) for n in range(N)], axis=0)
    return outs.astype(np.float32)


# revision 39
# speedup vs baseline: 1.2497x; 1.0507x over previous
"""Trainium2 Bass kernel for batched multi-head attention.

Problem: N=8, S=1024, E=1024, H=16, DK=64 MultiHeadAttention with a boolean
attention mask, fp32 reference.

Strategy: pure batch data-parallelism -- one batch element per NeuronCore
(8 cores), weights replicated, no collectives.  Per core everything is
computed in a transposed layout so no on-chip transposes are needed:

  xT [E, S] (host-transposed)  --Wq/Wk-->  QT, KT [E, S]  (no bias: the
      q-side bias and const cancel in softmax; the k-side bias folds into
      the exp bias B[s,h] = x[s]@(Wk_h bq_h)/8, precomputed on host)
  xT                           --Wv---->   V [S, E] head-major w/ ones col
  logitsT[k, q] = KT_h^T-slices @ QT_h    (K=64 row-paired matmuls on PE
      subarray tiles (0,0)/(64,0) -- two heads' matmuls run concurrently)
  Em = exp(logitsT/8 + B) * notm          (ACT exp w/ per-partition bias,
                                           DVE mask multiply; Em -> SBUF
                                           fp16 ring, 2 pairs deep)
  O_h[d|sum, q] = V_aug_h^T @ Em_h        (CLOSED 8-matmul accumulation
      bursts, delayed one pair: attnV for pair p runs during pair p+1's
      logits window.  Keeping accumulation groups closed around the 64-row
      logits matmuls avoids a ~10x PE penalty when >=2 PSUM groups are
      open across tile-size switches -- measured on HW.)
  oT[e', q] = O_h[0:64] * (1/sums)        (DVE; 1/sums broadcast across
      partitions via DRAM bounce; pair 7 uses a PE broadcast instead)
  out[q, e] = oT^T-slices @ Wo + bo_eff   (fp16 out, host casts to fp32)
"""

import numpy as np
from contextlib import ExitStack

import concourse.bass as bass
import concourse.mybir as mybir
import concourse.tile as tile
from concourse.tile_rust import add_dep_helper
from concourse.vector_clock import ScopedClock
from concourse.bass_utils import run_bass_kernel_spmd

F32 = mybir.dt.float32
F16 = mybir.dt.float16
Exp = mybir.ActivationFunctionType.Exp
MULT = mybir.AluOpType.mult

N, S, E, H, DK = 8, 1024, 1024, 16, 64
P = 128
NT = E // P
NPAIR = H // 2
DKP = DK + 1  # head slot width in V_aug (64 values + ones column)

MM_DT = F16


# ---------------------------------------------------------------------------
# Workaround: this walrus build supports at most ONE semaphore wait per
# instruction.  Split instructions carrying more waits into NOP(wait) chains
# on the same engine, and do the same for the TileContext tail drain.
# ---------------------------------------------------------------------------
_MAXW = int(__import__("os").environ.get("KMAXW", "1"))
_orig_lower = tile.TileContext._lower_ordered_insts
_tilefix_installed = False


def _split_waits(ordered):
    for _bb, insts in ordered.items():
        out = []
        for inst in insts:
            si = inst.sync_info
            if si is not None and len(si.on_wait) > _MAXW:
                waits = list(si.on_wait)
                keep, extra = waits[:_MAXW], waits[_MAXW:]
                for i in range(0, len(extra), _MAXW):
                    out.append(
                        mybir.InstNoOp(
                            name=f"{inst.name}-ws{i}",
                            engine=inst.engine,
                            bass_nofuse=True,
                            sync_info=mybir.SyncInfo(
                                on_wait=extra[i : i + _MAXW], on_update=[]
                            ),
                        )
                    )
                inst.sync_info = mybir.SyncInfo(
                    on_wait=keep, on_update=list(si.on_update)
                )
            out.append(inst)
        insts[:] = out


def _patched_lower(self, ordered):
    _split_waits(ordered)
    return _orig_lower(self, ordered)


def _patched_drain_and_barrier(self, tick_clock, wait_clock):
    nc = self.nc
    drain_inst = nc.sync.drain()
    wait_clock.add_sem_waits(
        drain_inst.ins, ScopedClock({None: tick_clock.global_clock})
    )
    si = drain_inst.ins.sync_info
    waits = list(si.on_wait) if si is not None else []
    if len(waits) > _MAXW:
        drain_inst.ins.sync_info = mybir.SyncInfo(on_wait=[], on_update=[])
        for i in range(0, len(waits), _MAXW):
            nop = nc.sync.nop(nofuse=True)
            nop.ins.sync_info = mybir.SyncInfo(
                on_wait=waits[i : i + _MAXW], on_update=[]
            )
    nc.all_engine_barrier()
    popped = nc._tile_sem_poison_stack.pop()
    assert popped is self._sem_poison
    nc.clear_and_free_semaphores(list(self.sems.allocated().values()))
    nc.all_engine_barrier()


def _install_tilefix():
    global _tilefix_installed
    if not _tilefix_installed:
        tile.TileContext._lower_ordered_insts = _patched_lower
        tile.TileContext._drain_and_barrier = _patched_drain_and_barrier
        _tilefix_installed = True


# ---------------------------------------------------------------------------
# Kernel build
# ---------------------------------------------------------------------------
_cached_nc = {}

# v-proj chunk placement: window -> list of (t, c) emitted as absorbers
_VMAP = {
    0: [(t, 0) for t in range(NT)],
    1: [(0, 1), (1, 1)],
    2: [(2, 1), (3, 1)],
    3: [(4, 1), (5, 1)],
    4: [(6, 1), (7, 1)],
    5: [],
    6: [],
    7: [],
}


def _build(repeat=1):
    if repeat in _cached_nc:
        return _cached_nc[repeat]
    _install_tilefix()

    nc = bass.Bass("TRN2", num_devices=N)

    x_t = nc.declare_dram_parameter("x_t", [E, S], MM_DT, isOutput=False)
    nm_t = nc.declare_dram_parameter("nm_t", [S, S], F16, isOutput=False)
    wq = nc.declare_dram_parameter("wq", [E, E], MM_DT, isOutput=False)
    wk = nc.declare_dram_parameter("wk", [E, E], MM_DT, isOutput=False)
    wv = nc.declare_dram_parameter("wv", [E, E], MM_DT, isOutput=False)
    wo = nc.declare_dram_parameter("wo", [E, E], MM_DT, isOutput=False)
    bt = nc.declare_dram_parameter("bt", [P, NT * H], F32, isOutput=False)
    bo_eff = nc.declare_dram_parameter("bo_eff", [E], F32, isOutput=False)
    out = nc.declare_dram_parameter("out", [S, E], F16, isOutput=True)
    import os as _os
    _dbg = _os.environ.get("KDEBUG") == "1"
    _peonly = _os.environ.get("KPEONLY") == "1"
    _exponly = False
    _expstage = _os.environ.get("KSTAGE") == "exp"
    _maskstage = _os.environ.get("KSTAGE") == "mask"
    if _dbg:
        qt0_d = nc.declare_dram_parameter("qt0_d", [P, S], MM_DT, isOutput=True)
        kt0_d = nc.declare_dram_parameter("kt0_d", [P, S], MM_DT, isOutput=True)
        em_d = nc.declare_dram_parameter("em_d", [P, S], F16, isOutput=True)
        o00_d = nc.declare_dram_parameter("o00_d", [DKP, S], F32, isOutput=True)
        rb00_d = nc.declare_dram_parameter("rb00_d", [DK, S], F32, isOutput=True)
        ot0_d = nc.declare_dram_parameter("ot0_d", [P, S], MM_DT, isOutput=True)
        vg0_d = nc.declare_dram_parameter("vg0_d", [P, H * DKP], F16, isOutput=True)

    def tiled(ap):
        return ap.rearrange("(t p) f -> p t f", p=P)

    x_tt = tiled(x_t.ap())
    nm_tt = tiled(nm_t.ap())
    w_t = {
        "q": tiled(wq.ap()),
        "k": tiled(wk.ap()),
        "v": tiled(wv.ap()),
        "o": tiled(wo.ap()),
    }
    bt_t = bt.ap().rearrange("p (t h) -> p t h", h=H)
    out_t = tiled(out.ap())

    with tile.TileContext(nc) as tc, ExitStack() as ctx:
        p_pers = ctx.enter_context(tc.tile_pool(name="pers", bufs=1))
        p_qk = ctx.enter_context(tc.tile_pool(name="qk", bufs=3))
        QT_p = {}
        KT_p = {}
        oT_t = [p_pers.tile([P, S], MM_DT, name=f"oT{j}") for j in range(NT)]
        nm_all = p_pers.tile([P, NT, S], F16, name="nm_all")
        nm_j = [nm_all[:, j, :] for j in range(NT)]
        Vg = p_pers.tile([P, NT, H * DKP], F16)
        BT_sb = p_pers.tile([P, NT, H], F32)
        ones1 = p_pers.tile([1, DK], F32)
        bo_sb = p_pers.tile([P, S], F32)

        for rep in range(repeat):
            with tc.tile_pool(name="em", bufs=(22 if _dbg else 24)) as p_em, \
                 tc.tile_pool(name="rr", bufs=2) as p_r, \
                 tc.tile_pool(name="rb", bufs=3) as p_rb, \
                 tc.tile_pool(name="oc", bufs=3) as p_oc, \
                 tc.tile_pool(name="po", bufs=2) as p_out, \
                 tc.tile_pool(name="dram", bufs=8, space="DRAM") as p_dram:

                QT_p.clear()
                KT_p.clear()
                nc.gpsimd.memset(Vg[:, :, DK::DKP], 1.0)
                nc.gpsimd.memset(ones1[:], 1.0)

                with tc.tile_pool(name="w", bufs=1) as p_w:
                    xT = p_w.tile([P, NT, S], MM_DT, name=f"xT_{rep}")
                    Wf = {
                        pr: p_w.tile([P, NT, S], MM_DT, name=f"Wf_{rep}_{pr}")
                        for pr in ("q", "k", "v")
                    }
                    # DMA priority order for cold start (batched: HWDGE
                    # costs ~630ns of serialized overhead PER instruction):
                    # SP queue: x + W t0-slices unblock pair-0 projections,
                    # then the Wq/Wk remainders.  ACT queue: mask, V, consts.
                    nc.sync.dma_start(xT[:], x_tt[:])
                    nc.sync.dma_start(Wf["q"][:, :, 0:P], w_t["q"][:, :, 0:P])
                    nc.sync.dma_start(Wf["k"][:, :, 0:P], w_t["k"][:, :, 0:P])
                    nc.scalar.dma_start(BT_sb[:], bt_t[:])
                    nc.scalar.dma_start(nm_all[:], nm_tt[:])
                    nc.sync.dma_start(Wf["q"][:, :, P:S], w_t["q"][:, :, P:S])
                    nc.sync.dma_start(Wf["k"][:, :, P:S], w_t["k"][:, :, P:S])
                    nc.scalar.dma_start(
                        Wf["v"][:, :, 0:512], w_t["v"][:, :, 0:512]
                    )
                    nc.scalar.dma_start(
                        Wf["v"][:, :, 512:1024], w_t["v"][:, :, 512:1024]
                    )
                    nc.scalar.dma_start(
                        bo_sb[:],
                        bo_eff.ap().rearrange("(o e) -> o e", o=1).broadcast_to((P, S)),
                    )

                    with tc.tile_pool(name="psO", bufs=1, space="PSUM") as psO:
                        psL = tc.alloc_tile_pool(name="psL", bufs=2, space="PSUM")

                        _pe_prev = [None]
                        _vg_copy = {}

                        def pe_mm(*args, chain=True, **kw):
                            # pin PE program order to emission order: the
                            # list scheduler otherwise hoists matmuls by
                            # readiness, splitting accumulation bursts
                            # around 64-row tiled matmuls (a ~10x HW
                            # penalty when >=2 PSUM groups are open across
                            # a tile-size switch) and racing ahead of Vg
                            # writers whose strided copies it won't track.
                            h = nc.tensor.matmul(*args, **kw)
                            if chain:
                                if _pe_prev[0] is not None:
                                    add_dep_helper(
                                        h.ins, _pe_prev[0], sync=False,
                                        reason="pe absorber-unit order",
                                    )
                                _pe_prev[0] = h.ins
                            return h

                        def emit_proj_chunk(proj, t, c):
                            acc = psL.tile(
                                [P, 512], F32, tag="A", bufs=2,
                                name=f"a2_{rep}_{proj}_{t}_{c}",
                            )
                            for j in range(NT):
                                wt = Wf[proj][:, j, :]
                                if proj == "v":
                                    lhsT = xT[:, j, t * P : (t + 1) * P]
                                    rhs = wt[:, c * 512 : (c + 1) * 512]
                                else:
                                    lhsT = wt[:, t * P : (t + 1) * P]
                                    rhs = xT[:, j, c * 512 : (c + 1) * 512]
                                pe_mm(
                                    acc[:], lhsT, rhs,
                                    start=(j == 0), stop=(j == NT - 1),
                                )
                            if proj == "v":
                                dst = Vg[
                                    :, t, c * 8 * DKP : (c + 1) * 8 * DKP
                                ].rearrange("p (h d) -> p h d", d=DKP)[:, :, 0:DK]
                                src = acc[:].rearrange("p (h d) -> p h d", d=DK)
                                cp = nc.vector.tensor_copy(dst, src)
                                _vg_copy[(t, c)] = cp.ins
                            else:
                                dst, tg = (
                                    (QT_p, "QT") if proj == "q" else (KT_p, "KT")
                                )
                                if t not in dst:
                                    dst[t] = p_qk.tile(
                                        [P, S], MM_DT, tag=tg,
                                        name=f"{tg}_{rep}_{t}",
                                    )
                                nc.vector.tensor_copy(
                                    dst[t][:, c * 512 : (c + 1) * 512], acc[:]
                                )

                        em_tiles = {}

                        def emit_logits_exp(p, j):
                            # four 1-bank L units per quartet; exps split per
                            # (h, c-half) so psL units release every ~570ns
                            # instead of per 2us exp pair (shrinks the
                            # PE<->ACT round-trip lockstep)
                            L = {}
                            for h in range(2):
                                L[h] = psL.tile(
                                    [P, S], F32, tag="L",
                                    name=f"L_{rep}_{p}_{j}_{h}",
                                )
                            for c in range(2):
                                for h in range(2):
                                    r0, r1 = h * DK, (h + 1) * DK
                                    pe_mm(
                                        L[h][:, c * 512 : (c + 1) * 512],
                                        KT_p[p][r0:r1, j * P : (j + 1) * P],
                                        QT_p[p][r0:r1, c * 512 : (c + 1) * 512],
                                        start=True, stop=True,
                                        tile_position=(r0, 0),
                                        chain=False,
                                    )
                            if _peonly:
                                for h in range(2):
                                    nc.vector.tensor_copy(
                                        oT_t[0][0:1, j * 16 : j * 16 + 16],
                                        L[h][0:1, 0:16],
                                    )
                                return
                            ems = {}
                            for h in range(2):
                                ems[h] = p_em.tile(
                                    [P, S], F16,
                                    tag=("Eme" if _expstage else "Em"),
                                    name=f"Em_{rep}_{p}_{j}_{h}",
                                    bufs=(3 if _expstage else None),
                                )
                                em_tiles[(p, j, h)] = ems[h]
                            import os as _os
                            for h in range(2):
                                _bias = (
                                    0.0 if _os.environ.get("KBIAS") == "0"
                                    else BT_sb[:, j, 2 * p + h : 2 * p + h + 1]
                                )
                                nc.scalar.activation(
                                    ems[h][:], L[h][:], Exp, scale=0.125,
                                    bias=_bias,
                                )
                            if _expstage:
                                nc.vector.tensor_copy(
                                    oT_t[1][0:1, 0:16], ems[0][0:1, 0:16],
                                )

                        def emit_mask(p, j):
                            if _expstage:
                                return
                            import os as _os
                            for h in range(2):
                                em = em_tiles[(p, j, h)]
                                eng = (
                                    nc.gpsimd
                                    if h == 0 and j in (1, 3, 5, 6)
                                    and _os.environ.get("KPOOLMASK") != "0"
                                    else nc.vector
                                )
                                eng.tensor_tensor(
                                    em[:], em[:], nm_j[j][:], MULT,
                                )
                                if _dbg and rep == 0 and (p, j, h) == (0, 0, 0):
                                    nc.sync.dma_start(em_d.ap(), em[:])

                        O_cur = {}
                        Oc_cur = {}

                        def emit_attn_burst(p, h, c):
                            # closed 8-matmul accumulation burst for pair p
                            if (p, h) not in O_cur:
                                O_cur[(p, h)] = psO.tile(
                                    [DKP, S], F32, tag="O",
                                    name=f"O_{rep}_{p}_{h}",
                                )
                            O = O_cur[(p, h)]
                            head = 2 * p + h
                            chalf = head // 8
                            jseq = [NT - 1] + list(range(NT - 1))
                            for i, j in enumerate(jseq):
                                rhs = (
                                    nm_j[j][:, c * 512 : (c + 1) * 512]
                                    if (_peonly or _expstage) else
                                    em_tiles[(p, j, h)][:, c * 512 : (c + 1) * 512]
                                )
                                mm = pe_mm(
                                    O[:, c * 512 : (c + 1) * 512],
                                    Vg[:, j, head * DKP : (head + 1) * DKP],
                                    rhs,
                                    start=(i == 0), stop=(i == NT - 1),
                                )
                                if i == 0:
                                    # the tile framework does not track the
                                    # strided Vg writes; sync the burst on
                                    # the last v-chunk copy of its c-half
                                    dep = _vg_copy.get((NT - 1, chalf))
                                    if dep is not None:
                                        add_dep_helper(
                                            mm.ins, dep, sync=True,
                                            reason="Vg strided-write race",
                                        )

                        def emit_norm_a(p, h):
                            if _peonly or _expstage or _maskstage:
                                nc.vector.tensor_copy(
                                    oT_t[p][0:DKP, 0:16],
                                    O_cur.pop((p, h))[:, 0:16],
                                )
                                return None
                            # reciprocal of both heads' softmax sums into one
                            # [2, S] tile; one DRAM-bounce broadcast to
                            # [2*DK, S] per pair (HWDGE instruction count)
                            O = O_cur[(p, h)]
                            if p == NPAIR - 1:
                                R7 = p_r.tile(
                                    [1, S], F32, tag="R", name=f"R7_{rep}_{h}"
                                )
                                nc.vector.reciprocal(R7[:], O[DK : DK + 1, :])
                                Oc = p_oc.tile(
                                    [DK, S], F16, tag="Oc",
                                    name=f"Oc_{rep}_{p}_{h}",
                                )
                                nc.vector.tensor_copy(Oc[:], O[0:DK, :])
                                O_cur.pop((p, h))
                                Oc_cur[(p, h)] = Oc
                                return R7
                            if h == 0:
                                R2 = p_r.tile(
                                    [2 * 32, S], F16, tag="R",
                                    name=f"R_{rep}_{p}",
                                )
                                emit_norm_a.r2 = R2
                            else:
                                R2 = emit_norm_a.r2
                            with nc.allow_low_precision(
                                reason="1/softmax-sum bounce in fp16; 5e-4 "
                                "rel err on normalized weights is fine"
                            ):
                                nc.vector.reciprocal(
                                    R2[32 * h : 32 * h + 1, :],
                                    O[DK : DK + 1, :],
                                )
                            Oc = p_oc.tile(
                                [DK, S], F16, tag="Oc",
                                name=f"Oc_{rep}_{p}_{h}",
                            )
                            nc.vector.tensor_copy(Oc[:], O[0:DK, :])
                            O_cur.pop((p, h))
                            Oc_cur[(p, h)] = Oc
                            if h == 0:
                                return None
                            Rd = p_dram.tile(
                                [2, S], F16, tag="Rd", name=f"Rd_{rep}_{p}"
                            )
                            nc.scalar.dma_start(
                                Rd[:],
                                R2[:].rearrange(
                                    "(a b) f -> a b f", b=32
                                )[:, 0:1, :],
                            )
                            Rbs = []
                            for hh in range(2):
                                Rbh = p_rb.tile(
                                    [DK, S], F16, tag="Rb",
                                    name=f"Rb_{rep}_{p}_{hh}",
                                )
                                nc.scalar.dma_start(
                                    Rbh[:],
                                    Rd[hh : hh + 1, :].broadcast_to((DK, S)),
                                )
                                Rbs.append(Rbh)
                            return Rbs

                        def emit_norm_b(p, h, Rb):
                            if _peonly or _expstage or _maskstage:
                                return
                            O = Oc_cur.pop((p, h))
                            Rbh = Rb[h] if isinstance(Rb, list) else Rb
                            nc.vector.tensor_tensor(
                                oT_t[p][h * DK : (h + 1) * DK, :],
                                O[0:DK, :], Rbh[:], MULT,
                            )

                        # ---- head: pair-0 q/k projections ----
                        for c in range(2):
                            emit_proj_chunk("q", 0, c)
                        for c in range(2):
                            emit_proj_chunk("k", 0, c)

                        if _dbg:
                            nc.sync.dma_start(qt0_d.ap(), QT_p[0][:])
                            nc.sync.dma_start(kt0_d.ap(), KT_p[0][:])
                        # ---- pair windows ----
                        norm_rb = {}
                        for p in range(NPAIR):
                            if p == NPAIR - 1:
                                # Wq is dead; reuse its SBUF for Wo.
                                nc.scalar.dma_start(
                                    Wf["q"][:], w_t["o"][:]
                                )
                            # absorber queue for this window
                            absq = []
                            if p > 0:
                                absq += [("burst", p - 1, 0, 0),
                                         ("burst", p - 1, 0, 1)]
                            if p < NPAIR - 1:
                                for c in range(2):
                                    absq.append(("chunk", "q", p + 1, c))
                            if p > 0:
                                absq += [("burst", p - 1, 1, 0),
                                         ("burst", p - 1, 1, 1)]
                            if p < NPAIR - 1:
                                for c in range(2):
                                    absq.append(("chunk", "k", p + 1, c))
                            for t, c in _VMAP[p]:
                                absq.append(("chunk", "v", t, c))
                            # interleave: spread absorbers evenly across the
                            # 8 quartet slots (emit after each quartet until
                            # the backlog matches the remaining slots)
                            total = len(absq)

                            def pop_abs():
                                op = absq.pop(0)
                                if op[0] == "burst":
                                    _, bp, bh, bc = op
                                    emit_attn_burst(bp, bh, bc)
                                    if bc == 1:
                                        rbv = emit_norm_a(bp, bh)
                                        if rbv is not None:
                                            norm_rb[bp] = rbv
                                else:
                                    _, pr, t, c = op
                                    emit_proj_chunk(pr, t, c)

                            for j in range(NT):
                                emit_logits_exp(p, j)
                                want_done = (total * (j + 1) + NT - 1) // NT
                                while absq and total - len(absq) < want_done:
                                    pop_abs()
                                if j >= 2:
                                    emit_mask(p, j - 2)
                            while absq:
                                pop_abs()
                            emit_mask(p, NT - 2)
                            emit_mask(p, NT - 1)
                            if p > 0 and not (_peonly or _expstage or _maskstage):
                                rbv = norm_rb.pop(p - 1)
                                for hh in range(2):
                                    emit_norm_b(p - 1, hh, rbv)


                        if _dbg:
                            nc.sync.dma_start(ot0_d.ap(), oT_t[0][:])
                            nc.sync.dma_start(vg0_d.ap(), Vg[:, 0, :])
                        # ---- tail: attnV(7), PE-broadcast norm, out-proj ----
                        R_hist = {}
                        for h, c in ((0, 0), (0, 1), (1, 0), (1, 1)):
                            emit_attn_burst(NPAIR - 1, h, c)
                            if c == 1:
                                R_hist[h] = emit_norm_a(NPAIR - 1, h)
                        psL.release()
                        with tc.tile_pool(name="psC", bufs=2, space="PSUM") as psC:
                            p7 = NPAIR - 1
                            if _peonly or _expstage or _maskstage:
                                nc.gpsimd.memset(oT_t[0][:], 0.5)
                                nc.gpsimd.memset(oT_t[1][:, 16:S], 0.5)
                                for jj in range(1, NT):
                                    nc.gpsimd.memset(oT_t[jj][:], 0.5)
                            for h in range(2 * (0 if (_peonly or _expstage or _maskstage) else 1)):
                                Rp = psC.tile(
                                    [DK, S], F32, tag="F", name=f"Rp_{rep}_{h}"
                                )
                                for c in range(2):
                                    pe_mm(
                                        Rp[:, c * 512 : (c + 1) * 512],
                                        ones1[:],
                                        R_hist[h][:, c * 512 : (c + 1) * 512],
                                        start=True, stop=True,
                                    )
                                Rs = p_rb.tile(
                                    [DK, S], F32, tag="Rb", name=f"Rs_{rep}_{h}"
                                )
                                nc.vector.tensor_copy(Rs[:], Rp[:])
                                emit_norm_b(p7, h, Rs)
                            for t in range(NT):
                                F = psC.tile(
                                    [P, S], F32, tag="F", name=f"F_{rep}_{t}"
                                )
                                for j in range(NT):
                                    for c in range(2):
                                        pe_mm(
                                            F[:, c * 512 : (c + 1) * 512],
                                            oT_t[j][:, t * P : (t + 1) * P],
                                            Wf["q"][:, j, c * 512 : (c + 1) * 512],
                                            start=(j == 0), stop=(j == NT - 1),
                                        )
                                ot = p_out.tile(
                                    [P, S], F16, tag="ot", name=f"ot_{rep}_{t}"
                                )
                                nc.vector.tensor_add(ot[:], F[:], bo_sb[:])
                                nc.sync.dma_start(out_t[:, t, :], ot[:])

    _cached_nc[repeat] = nc
    return nc


# ---------------------------------------------------------------------------
# Entry point
# ---------------------------------------------------------------------------
def make_in_maps(x, attn_mask, Wq, bq, Wk, bk, Wv, bv, Wo, bo):
    ndt = np.float16
    Wk64 = np.asarray(Wk, np.float64)
    bq64 = np.asarray(bq, np.float64)
    # per-head exp bias direction: wb[:, h] = (Wk_h @ bq_h) / 8
    WB = np.stack(
        [
            Wk64[:, h * DK : (h + 1) * DK] @ bq64[h * DK : (h + 1) * DK] / 8.0
            for h in range(H)
        ],
        axis=1,
    )  # [E, H]
    bo_eff = (
        np.asarray(bv, np.float64) @ np.asarray(Wo, np.float64)
        + np.asarray(bo, np.float64)
    ).astype(np.float32)
    wqc = np.asarray(Wq, np.float32).astype(ndt)
    wkc = np.asarray(Wk, np.float32).astype(ndt)
    wvc = np.asarray(Wv, np.float32).astype(ndt)
    woc = np.asarray(Wo, np.float32).astype(ndt)
    in_maps = []
    for n in range(N):
        notm_t = np.ascontiguousarray(
            (1.0 - np.asarray(attn_mask[n], np.float32)).T
        ).astype(np.float16)
        B = np.asarray(x[n], np.float64) @ WB  # [S, H]
        btc = np.ascontiguousarray(
            B.reshape(NT, P, H).transpose(1, 0, 2).reshape(P, NT * H)
        ).astype(np.float32)
        in_maps.append(
            {
                "x_t": np.ascontiguousarray(np.asarray(x[n], np.float32).T).astype(ndt),
                "nm_t": notm_t,
                "wq": wqc, "wk": wkc, "wv": wvc, "wo": woc,
                "bt": btc, "bo_eff": bo_eff,
            }
        )
    return in_maps


def kernel(x, attn_mask, Wq, bq, Wk, bk, Wv, bv, Wo, bo, **_):
    nc = _build()
    in_maps = make_in_maps(x, attn_mask, Wq, bq, Wk, bk, Wv, bv, Wo, bo)
    res = run_bass_kernel_spmd(nc, in_maps, list(range(N)))
    outs = np.stack([np.asarray(res.results[n]["out"]) for n in range(N)], axis=0)
    return outs.astype(np.float32)
